# revision 1
# baseline (speedup 1.0000x reference)
"""AttentiveStatPooling Trainium2 kernel (8-core SPMD, data-parallel over batch).

Contract: kernel(**inputs) takes the FULL unsharded inputs (as produced by
reference.setup_inputs()) and returns the FULL [B, 2C] output.

Math (per sample, identical to the jax reference):
  mean/std over T of x;  h = relu(Wx@x + (Wm@mean + Ws@std + b1));
  g = tanh(BN1(h));  l = BN2scale * relu(W2@g + b2)  (the BN2 shift cancels in
  the softmax and is dropped);  w = softmax(l, axis=T);
  out = [sum(x*w), sqrt(clip(sum(x^2*w) - mu^2, 1e-4))].

Implementation notes:
  - batch 32 split 4 samples/core across 8 NeuronCores (pure DP).
  - x shipped in bf16 (halves DMA; matmuls/weighted sums read bf16, all
    reductions accumulate fp32 on-engine via accum_out).
  - BN affines folded into per-partition ACT scale/bias vectors (host-side).
  - softmax needs no max-subtraction (logits bounded, per-row shift cancels);
    relu inside the softmax realized as max(exp(l), 1).
  - sqrt via Newton/rsqrt on the vector engine (avoids ACT table switches).
  - emission is software-pipelined: phase A of sample s+2 and phase B of
    sample s+1 are interleaved into phase C of sample s so every engine's
    in-order instruction stream stays busy.
"""

import numpy as np
import ml_dtypes

B, C, T, A = 32, 1536, 1000, 128
N_CORES = 8
SPC = B // N_CORES        # samples per core
NCH = C // 128            # 12 channel chunks of 128
BN_EPS = 1e-5
CLAMP = 1e-4
HALVES = ((0, 512), (512, 1000))   # psum-bank-aligned split of T

_CACHE = {}


def _build_module(loop_reps=1):
    import concourse.tile as tile
    from concourse import bacc, mybir
    from contextlib import ExitStack

    f32, bf16 = mybir.dt.float32, mybir.dt.bfloat16
    Alu = mybir.AluOpType
    Act = mybir.ActivationFunctionType

    nc = bacc.Bacc("TRN2", target_bir_lowering=False, debug=False,
                   num_devices=N_CORES)

    xbf = nc.dram_tensor("xbf", [SPC, C, T], bf16, kind="ExternalInput").ap()
    w1xT = nc.dram_tensor("w1xT", [C, A], bf16, kind="ExternalInput").ap()
    wmsT = nc.dram_tensor("wmsT", [2 * C, A], f32, kind="ExternalInput").ap()
    w2T = nc.dram_tensor("w2T", [A, C], bf16, kind="ExternalInput").ap()
    b1d = nc.dram_tensor("b1d", [A, 1], f32, kind="ExternalInput").ap()
    inv1d = nc.dram_tensor("inv1d", [A, 1], f32, kind="ExternalInput").ap()
    add1d = nc.dram_tensor("add1d", [A, 1], f32, kind="ExternalInput").ap()
    inv2d = nc.dram_tensor("inv2d", [128, NCH], f32, kind="ExternalInput").ap()
    b2pd = nc.dram_tensor("b2pd", [128, NCH], f32, kind="ExternalInput").ap()
    identd = nc.dram_tensor("identd", [128, 128], f32, kind="ExternalInput").ap()
    out = nc.dram_tensor("out", [SPC, 2 * C], f32, kind="ExternalOutput").ap()

    with tile.TileContext(nc) as tc:
        with ExitStack() as ctx:
            cpool = ctx.enter_context(tc.tile_pool(name="const", bufs=1))
            xpool = ctx.enter_context(tc.tile_pool(name="x", bufs=14))
            epool = ctx.enter_context(tc.tile_pool(name="e", bufs=3))
            ebpool = ctx.enter_context(tc.tile_pool(name="eb", bufs=3))
            ppool = ctx.enter_context(tc.tile_pool(name="p", bufs=3))
            jpool = ctx.enter_context(tc.tile_pool(name="junk", bufs=6))
            rpool = ctx.enter_context(tc.tile_pool(name="r", bufs=2))
            gpool = ctx.enter_context(tc.tile_pool(name="g", bufs=2))
            spool = ctx.enter_context(tc.tile_pool(name="stats", bufs=3))
            smpool = ctx.enter_context(tc.tile_pool(name="small", bufs=8))
            opool = ctx.enter_context(tc.tile_pool(name="ostage", bufs=4))
            ph1p = ctx.enter_context(tc.tile_pool(name="ph1", bufs=1, space="PSUM"))
            p2p = ctx.enter_context(tc.tile_pool(name="p2", bufs=2, space="PSUM"))
            pmvp = ctx.enter_context(tc.tile_pool(name="pmv", bufs=1, space="PSUM"))
            ptrp = ctx.enter_context(tc.tile_pool(name="ptr", bufs=1, space="PSUM"))

            st = {}   # per-sample state

            def dma_x(s, groups=range(4)):
                if s not in st:
                    st[s] = {"xg": [], "x": []}
                for g in groups:
                    xt = xpool.tile([128, 3 * T], bf16, name="x", tag="x")
                    src_ap = xbf[s, g * 384:(g + 1) * 384, :]
                    src_ap = src_ap.rearrange("(c p) t -> p c t", p=128)
                    nc.sync.dma_start(xt[:].rearrange("p (c t) -> p c t", t=T), src_ap)
                    st[s]["xg"].append(xt)
                    for i in range(3):
                        st[s]["x"].append(xt[:, i * T:(i + 1) * T])

            def phaseA_moments(s, c):
                """mean/var of chunk c of sample s via bn_stats."""
                d = st[s]
                if c == 0:
                    d["msv"] = spool.tile([128, 2 * NCH], f32, name="msv", tag="msv")
                xt = d["x"][c]
                bnst = smpool.tile([128, 12], f32, name="bnst", tag="bnst")
                nc.vector.bn_stats(bnst[:, 0:6], xt[:, 0:500])
                nc.vector.bn_stats(bnst[:, 6:12], xt[:, 500:1000])
                nc.vector.bn_aggr(d["msv"][:, 2 * c:2 * c + 2], bnst[:])

            def phaseA_mm1(s, c):
                d = st[s]
                if c == 0:
                    d["ph1"] = ph1p.tile([A, T], f32, name="ph1", tag="ph1")
                xt = d["x"][c]
                for lo, hi in HALVES:
                    nc.tensor.matmul(d["ph1"][:, lo:hi], w1xT_t[c],
                                     xt[:, lo:hi], start=(c == 0),
                                     stop=(c == NCH - 1), skip_group_check=True)

            def newton_rsqrt(v_ap, out_ap, n, iters):
                """out = 1/sqrt(v) elementwise on a [128, n] fp32 AP."""
                t0 = smpool.tile([128, n], f32, name="nw0", tag="nw0")
                t1 = smpool.tile([128, n], f32, name="nw1", tag="nw1")
                r = smpool.tile([128, n], f32, name="nwr", tag="nwr")
                nc.vector.tensor_scalar(t0[:], v_ap, 0.5, 0.5, Alu.mult, Alu.add)
                nc.vector.reciprocal(r[:], t0[:])
                for it in range(iters):
                    dst = out_ap if it == iters - 1 else r[:]
                    nc.vector.tensor_tensor(t0[:], v_ap, r[:], Alu.mult)
                    nc.vector.tensor_tensor(t1[:], t0[:], r[:], Alu.mult)
                    nc.vector.tensor_scalar(t0[:], t1[:], -0.5, 1.5, Alu.mult, Alu.add)
                    nc.vector.tensor_tensor(dst, r[:], t0[:], Alu.mult)

            def phaseB_stats(s):
                """mean/std + mean-half of the bias matvec."""
                d = st[s]
                msv = d["msv"]
                mean_cols = msv[:].rearrange("p (c two) -> p c two", two=2)[:, :, 0]
                var_cols = msv[:].rearrange("p (c two) -> p c two", two=2)[:, :, 1]
                pmv = pmvp.tile([A, 1], f32, name="pmv", tag="pmv")
                d["pmv"] = pmv
                for k in range(NCH):
                    nc.tensor.matmul(pmv[:], wms_t[k], msv[:, 2 * k:2 * k + 1],
                                     start=(k == 0), stop=False,
                                     skip_group_check=True)
                std_t = smpool.tile([128, NCH], f32, name="std_t", tag="std_t")
                d["std_t"] = std_t
                v = smpool.tile([128, NCH], f32, name="v", tag="v")
                nc.vector.tensor_scalar(v[:], var_cols, T / (T - 1.0), CLAMP,
                                        Alu.mult, Alu.max)
                rs = smpool.tile([128, NCH], f32, name="rs", tag="rs")
                newton_rsqrt(v[:], rs[:], NCH, 3)
                nc.vector.tensor_tensor(std_t[:], v[:], rs[:], Alu.mult)

            def phaseB_main(s):
                """std-half of the matvec -> relu -> tanh (g)."""
                d = st[s]
                pmv, std_t = d["pmv"], d["std_t"]
                for k in range(NCH):
                    nc.tensor.matmul(pmv[:], wms_t[NCH + k], std_t[:, k:k + 1],
                                     start=False, stop=(k == NCH - 1),
                                     skip_group_check=True)
                btot = smpool.tile([A, 1], f32, name="btot", tag="btot")
                nc.vector.tensor_tensor(btot[:], pmv[:], b1_t[:], Alu.add)
                rt = rpool.tile([A, T], f32, name="r", tag="r")
                nc.scalar.activation(rt[:], d["ph1"][:], Act.Relu, bias=btot[:])
                gt = gpool.tile([A, T], bf16, name="g", tag="g")
                nc.scalar.activation(gt[:], rt[:], Act.Tanh, bias=add1_t[:],
                                     scale=inv1_t[:])
                d["g"] = gt

            def phaseB(s):
                phaseB_stats(s)
                phaseB_main(s)

            def phaseC_chunk(s, c):
                d = st[s]
                if c == 0:
                    d["S0"] = spool.tile([128, NCH], f32, name="S0", tag="S0")
                    d["S1"] = spool.tile([128, NCH], f32, name="S1", tag="S1")
                    d["S2"] = spool.tile([128, NCH], f32, name="S2", tag="S2")
                p2 = p2p.tile([128, T], f32, name="p2", tag="p2")
                wsl = w2T_t[:, c * 128:(c + 1) * 128]
                for lo, hi in HALVES:
                    nc.tensor.matmul(p2[:, lo:hi], wsl, d["g"][:, lo:hi],
                                     start=True, stop=True)
                E = epool.tile([128, T], bf16, name="E", tag="E")
                nc.scalar.activation(E[:], p2[:], Act.Exp,
                                     bias=b2p_t[:, c:c + 1], scale=inv2_t[:, c:c + 1])
                eb = ebpool.tile([128, T], bf16, name="eb", tag="eb")
                nc.vector.tensor_scalar(eb[:], E[:], 1.0, None, Alu.max)
                j0 = jpool.tile([128, T], bf16, name="junk", tag="junk")
                nc.scalar.activation(j0[:], eb[:], Act.Identity,
                                     accum_out=d["S0"][:, c:c + 1])
                xt = d["x"][c]
                pt = ppool.tile([128, T], bf16, name="p", tag="p")
                nc.vector.tensor_tensor(pt[:], eb[:], xt, Alu.mult)
                j1 = jpool.tile([128, T], bf16, name="junk", tag="junk")
                nc.vector.tensor_scalar(j1[:], pt[:], 0.0, 0.0, Alu.add,
                                        Alu.add, accum_out=d["S1"][:, c:c + 1])
                qt = ppool.tile([128, T], bf16, name="p", tag="p")
                nc.vector.tensor_tensor(qt[:], pt[:], xt, Alu.mult)
                j2 = jpool.tile([128, T], bf16, name="junk", tag="junk")
                nc.scalar.activation(j2[:], qt[:], Act.Identity,
                                     accum_out=d["S2"][:, c:c + 1])

            def sample_out(s):
                """mu/sg + transpose (DVE 32x32 blocks) + store."""
                d = st[s]
                rc = smpool.tile([128, NCH], f32, name="rc", tag="rc")
                nc.vector.reciprocal(rc[:], d["S0"][:])
                mu = opool.tile([128, NCH], f32, name="mu", tag="mu")
                sg = opool.tile([128, NCH], f32, name="sg", tag="sg")
                nc.vector.tensor_tensor(mu[:], d["S1"][:], rc[:], Alu.mult)
                ex2 = smpool.tile([128, NCH], f32, name="ex2", tag="ex2")
                nc.vector.tensor_tensor(ex2[:], d["S2"][:], rc[:], Alu.mult)
                mu2 = smpool.tile([128, NCH], f32, name="mu2", tag="mu2")
                nc.vector.tensor_tensor(mu2[:], mu[:], mu[:], Alu.mult)
                sg2 = smpool.tile([128, NCH], f32, name="sg2", tag="sg2")
                nc.vector.scalar_tensor_tensor(sg2[:], mu2[:], -1.0, ex2[:],
                                               Alu.mult, Alu.add)
                v2 = smpool.tile([128, NCH], f32, name="v2", tag="v2")
                nc.vector.tensor_scalar(v2[:], sg2[:], 1.0, CLAMP, Alu.mult, Alu.max)
                rsg = smpool.tile([128, NCH], f32, name="rsg", tag="rsg")
                newton_rsqrt(v2[:], rsg[:], NCH, 6)
                nc.vector.tensor_tensor(sg[:], v2[:], rsg[:], Alu.mult)
                for half, srct in ((0, mu), (1, sg)):
                    ptr = ptrp.tile([NCH, 128], f32, name="ptr", tag="ptr")
                    nc.tensor.transpose(ptr[:], srct[:], ident_t[:])
                    ost = opool.tile([NCH, 128], f32, name="ost", tag="ost")
                    nc.scalar.copy(ost[:], ptr[:])
                    dst = out[s, half * C:(half + 1) * C]
                    dst = dst.rearrange("(ci p) -> ci p", p=128)
                    nc.sync.dma_start(dst, ost[:])

            # ---------------- constant loads (interleaved with x below) ----
            def load_w1xT():
                t = cpool.tile([128, NCH * A], bf16, name="w1xall", tag="w1xall")
                src_ap = w1xT.rearrange("(c p) a -> p c a", p=128)
                nc.sync.dma_start(t[:].rearrange("p (c a) -> p c a", a=A), src_ap)
                return [t[:, c * A:(c + 1) * A] for c in range(NCH)]

            def load_params():
                global b1_t, inv1_t, add1_t, inv2_t, b2p_t, w2T_t, wms_t, ident_t
                b1_t = cpool.tile([A, 1], f32, name="b1", tag="b1")
                nc.sync.dma_start(b1_t[:], b1d[:])
                inv1_t = cpool.tile([A, 1], f32, name="inv1", tag="inv1")
                nc.sync.dma_start(inv1_t[:], inv1d[:])
                add1_t = cpool.tile([A, 1], f32, name="add1", tag="add1")
                nc.sync.dma_start(add1_t[:], add1d[:])
                inv2_t = cpool.tile([128, NCH], f32, name="inv2", tag="inv2")
                nc.sync.dma_start(inv2_t[:], inv2d[:])
                b2p_t = cpool.tile([128, NCH], f32, name="b2p", tag="b2p")
                nc.sync.dma_start(b2p_t[:], b2pd[:])
                w2T_t = cpool.tile([A, C], bf16, name="w2T", tag="w2T")
                nc.sync.dma_start(w2T_t[:], w2T[:])
                ident_t = cpool.tile([128, 128], f32, name="ident", tag="ident")
                nc.sync.dma_start(ident_t[:], identd[:])
                wt = cpool.tile([128, 2 * NCH * A], f32, name="wmsall", tag="wmsall")
                src_ap = wmsT.rearrange("(k p) a -> p k a", p=128)
                nc.sync.dma_start(wt[:].rearrange("p (k a) -> p k a", a=A), src_ap)
                wms_t = [wt[:, k * A:(k + 1) * A] for k in range(2 * NCH)]

            def body():
                global w1xT_t
                # prologue: phase A of samples 0/1, weights interleaved,
                # sample 2's DMA prefetched
                dma_x(0, groups=[0])
                w1xT_t = load_w1xT()
                dma_x(0, groups=[1, 2, 3])
                for c in range(NCH):
                    phaseA_moments(0, c)
                    phaseA_mm1(0, c)
                load_params()
                dma_x(1)
                for c in range(NCH):
                    phaseA_moments(1, c)
                dma_x(2)
                phaseB(0)
                for c in range(NCH):
                    phaseA_mm1(1, c)
                # steady state: C(s) carries A(s+2) moments, B(s+1) at c6,
                # and A(s+2)'s matmul1 interleaved in the c>=7 shadow of
                # relu(s+1) freeing the ph1 slot.
                for s in range(SPC):
                    for c in range(NCH):
                        phaseC_chunk(s, c)
                        if s + 3 < SPC and c == 0:
                            dma_x(s + 3)
                        if s + 2 < SPC and c < 8:
                            phaseA_moments(s + 2, c)
                        if s + 2 < SPC and c >= 8:
                            phaseA_moments(s + 2, c)
                            for cc in range(3 * (c - 8), 3 * (c - 8) + 3):
                                phaseA_mm1(s + 2, cc)
                        if c == 3 and s + 1 < SPC:
                            phaseB_stats(s + 1)
                        if c == 5 and s + 1 < SPC:
                            phaseB_main(s + 1)
                    sample_out(s)
                    del st[s]

            if loop_reps == 1:
                body()
            else:
                with tc.For_i(0, loop_reps, 1):
                    body()

    nc.compile()
    return nc


def _get_module(loop_reps=1):
    key = loop_reps
    if key not in _CACHE:
        _CACHE[key] = _build_module(loop_reps)
    return _CACHE[key]


def _host_prep(inputs):
    """Precompute folded parameters and shard inputs. Returns per-core in_maps."""
    x = np.asarray(inputs["x"])
    W1 = np.asarray(inputs["W1"], np.float32)
    b1 = np.asarray(inputs["b1"], np.float32)
    g1 = np.asarray(inputs["g1"], np.float32)
    beta1 = np.asarray(inputs["beta1"], np.float32)
    rm1 = np.asarray(inputs["rm1"], np.float32)
    rv1 = np.asarray(inputs["rv1"], np.float32)
    W2 = np.asarray(inputs["W2"], np.float32)
    b2 = np.asarray(inputs["b2"], np.float32)
    g2 = np.asarray(inputs["g2"], np.float32)
    rv2 = np.asarray(inputs["rv2"], np.float32)

    inv1 = (g1 / np.sqrt(rv1 + BN_EPS)).astype(np.float32)
    add1 = (beta1 - rm1 * inv1).astype(np.float32)
    inv2 = (g2 / np.sqrt(rv2 + BN_EPS)).astype(np.float32)
    b2p = (inv2 * b2).astype(np.float32)

    const = {
        "w1xT": np.ascontiguousarray(W1[:, :C].T).astype(ml_dtypes.bfloat16),
        "wmsT": np.ascontiguousarray(W1[:, C:].T).astype(np.float32),
        "w2T": np.ascontiguousarray(W2.T).astype(ml_dtypes.bfloat16),
        "b1d": b1.reshape(A, 1),
        "inv1d": inv1.reshape(A, 1),
        "add1d": add1.reshape(A, 1),
        "inv2d": np.ascontiguousarray(inv2.reshape(NCH, 128).T),
        "b2pd": np.ascontiguousarray(b2p.reshape(NCH, 128).T),
        "identd": np.eye(128, dtype=np.float32),
    }
    xbf = x.astype(ml_dtypes.bfloat16)
    in_maps = []
    for core in range(N_CORES):
        m = dict(const)
        m["xbf"] = np.ascontiguousarray(xbf[core * SPC:(core + 1) * SPC])
        in_maps.append(m)
    return in_maps


def kernel(**inputs):
    from concourse.bass_utils import run_bass_kernel_spmd

    nc = _get_module(loop_reps=1)
    in_maps = _host_prep(inputs)
    res = run_bass_kernel_spmd(nc, in_maps, core_ids=list(range(N_CORES)))
    out = np.concatenate([res.results[i]["out"] for i in range(N_CORES)], axis=0)
    return out.astype(np.float32)



# revision 36
# speedup vs baseline: 1217.2824x; 1217.2824x over previous
"""AttentiveStatPooling Trainium2 kernel (8-core SPMD, data-parallel over batch).

Contract: kernel(**inputs) takes the FULL unsharded inputs (as produced by
reference.setup_inputs()) and returns the FULL [B, 2C] output.

Math (per sample, identical to the jax reference):
  mean/std over T of x;  h = relu(Wx@x + (Wm@mean + Ws@std + b1));
  g = tanh(BN1(h));  l = BN2scale * relu(W2@g + b2)  (the BN2 shift cancels in
  the softmax and is dropped);  w = softmax(l, axis=T);
  out = [sum(x*w), sqrt(clip(sum(x^2*w) - mu^2, 1e-4))].

Implementation notes (v3 — four-engine balance):
  - batch 32 split 4 samples/core across 8 NeuronCores (pure DP).
  - x shipped in bf16 (halves DMA; all elementwise work runs in bf16 so the
    DVE hits its 2x (tensor_tensor) / 4x (tensor_scalar) perf modes).
  - per-chunk softmax stats with fused accumulates (no standalone reduce):
      eb = max(E,1)  tensor_scalar(max) 4x, fused accum -> S0   [DVE]
      pt = eb*x      tensor_tensor 2x                           [DVE]
      S1 = sum(pt)   tensor_scalar(id) 4x + accum               [DVE or ACT]
      qt = pt*x      tensor_tensor                              [Pool or DVE]
      S2 = sum(qt)   tensor_scalar(id) 4x + accum               [DVE]
    The qt multiply rides the otherwise-idle GPSIMD/Pool engine (plain
    tensor_tensor is the only elementwise op walrus allows there); its S2
    accumulate is deferred two chunks so the DVE never waits on the Pool.
  - x moments per chunk: sum(x) on DVE (tensor_scalar 4x + accum),
    sum(x^2) on ACT (Square+accum) / DVE (x*x + accum) per the *_SQ maps.
    All ACT functions used (Exp/Tanh/Square/Relu/Identity) live in one
    activation table set, so there are no table reloads.
  - softmax needs no max-subtraction (logits bounded, per-row shift cancels);
    relu inside the softmax realized as max(exp(l), 1).
  - sqrt via Newton/rsqrt on the vector engine (avoids ACT table switches),
    3 DVE ops per iteration via scalar_tensor_tensor.
  - emission is software-pipelined: mm2+exp run two chunks ahead of the DVE
    consumer, phase A of sample s+2 and phase B of sample s+1 interleave
    into phase C of sample s, and the previous sample's S2 flush + output
    stage defer into the next sample's chunk stream.  The ENGINE_* knob
    strings balance per-chunk work across DVE/ACT/Pool for the "front"
    samples (which carry phase-A work) and the "tail" samples (which don't).
"""

import numpy as np
import ml_dtypes

B, C, T, A = 32, 1536, 1000, 128
N_CORES = 8
SPC = B // N_CORES        # samples per core
NCH = C // 128            # 12 channel chunks of 128
BN_EPS = 1e-5
CLAMP = 1e-4
HALVES = ((0, 512), (512, 1000))   # psum-bank-aligned split of T

_CACHE = {}

# Engine-balance knobs (index = c % 12).  For each chunk of "front" samples
# (those that also carry phase-A work of sample s+2) and "tail" samples:
#   qt engine: 'P' (Pool) or 'D' (DVE)
#   S1 accumulate: 'D' (DVE tensor_scalar) or 'A' (ACT identity+accum)
#   phase-A sum(x^2): 'A' (ACT Square), 'D' (DVE x*x+accum), 'P' (Pool x*x
#   + DVE accum)
FRONT_QT = "PPPPPPPPPPPP"
FRONT_S1 = "DDDDDDDDDDDD"
FRONT_SQ = "AAAAAAAAAAAD"
TAIL_QT = "DPPPDPPPDPPP"
TAIL_S1 = "ADADADADADAD"
PRO_SQ = "DADADADADADA"
FRONT_MX = "DDDDDDDDDDDD"   # prologue samples 0/1 (rotated by 6 for s=1)


def _build_module(loop_reps=1):
    import concourse.tile as tile
    from concourse import bacc, mybir
    from contextlib import ExitStack

    f32, bf16 = mybir.dt.float32, mybir.dt.bfloat16
    Alu = mybir.AluOpType
    Act = mybir.ActivationFunctionType

    nc = bacc.Bacc("TRN2", target_bir_lowering=False, debug=False,
                   num_devices=N_CORES)

    xbf = nc.dram_tensor("xbf", [SPC, C, T], bf16, kind="ExternalInput").ap()
    w1xT = nc.dram_tensor("w1xT", [C, A], bf16, kind="ExternalInput").ap()
    wmsT = nc.dram_tensor("wmsT", [2 * C, A], f32, kind="ExternalInput").ap()
    w2T = nc.dram_tensor("w2T", [A, C], bf16, kind="ExternalInput").ap()
    b1d = nc.dram_tensor("b1d", [A, 1], f32, kind="ExternalInput").ap()
    inv1d = nc.dram_tensor("inv1d", [A, 1], f32, kind="ExternalInput").ap()
    add1d = nc.dram_tensor("add1d", [A, 1], f32, kind="ExternalInput").ap()
    inv2d = nc.dram_tensor("inv2d", [128, NCH], f32, kind="ExternalInput").ap()
    b2pd = nc.dram_tensor("b2pd", [128, NCH], f32, kind="ExternalInput").ap()
    identd = nc.dram_tensor("identd", [128, 128], f32, kind="ExternalInput").ap()
    out = nc.dram_tensor("out", [SPC, 2 * C], f32, kind="ExternalOutput").ap()

    with tile.TileContext(nc) as tc:
        with ExitStack() as ctx:
            cpool = ctx.enter_context(tc.tile_pool(name="const", bufs=1))
            xpool = ctx.enter_context(tc.tile_pool(name="x", bufs=14))
            epool = ctx.enter_context(tc.tile_pool(name="e", bufs=3))
            ebpool = ctx.enter_context(tc.tile_pool(name="eb", bufs=3))
            ppool = ctx.enter_context(tc.tile_pool(name="p", bufs=3))
            qpool = ctx.enter_context(tc.tile_pool(name="q", bufs=4))
            jpool = ctx.enter_context(tc.tile_pool(name="junk", bufs=6))
            rpool = ctx.enter_context(tc.tile_pool(name="r", bufs=2))
            gpool = ctx.enter_context(tc.tile_pool(name="g", bufs=2))
            spool = ctx.enter_context(tc.tile_pool(name="stats", bufs=4))
            smpool = ctx.enter_context(tc.tile_pool(name="small", bufs=8))
            opool = ctx.enter_context(tc.tile_pool(name="ostage", bufs=4))
            ph1p = ctx.enter_context(tc.tile_pool(name="ph1", bufs=1, space="PSUM"))
            p2p = ctx.enter_context(tc.tile_pool(name="p2", bufs=2, space="PSUM"))
            pmvp = ctx.enter_context(tc.tile_pool(name="pmv", bufs=1, space="PSUM"))
            ptrp = ctx.enter_context(tc.tile_pool(name="ptr", bufs=1, space="PSUM"))

            st = {}   # per-sample state

            def dma_x(s, groups=range(4), split_first=False):
                if s not in st:
                    st[s] = {"xg": [], "x": []}
                for g in groups:
                    xt = xpool.tile([128, 3 * T], bf16, name="x", tag="x")
                    src_ap = xbf[s, g * 384:(g + 1) * 384, :]
                    src_ap = src_ap.rearrange("(c p) t -> p c t", p=128)
                    dst = xt[:].rearrange("p (c t) -> p c t", t=T)
                    if split_first and g == 0:
                        # land chunk 0 first so phase A can start sooner
                        nc.sync.dma_start(dst[:, 0:1], src_ap[:, 0:1])
                        nc.sync.dma_start(dst[:, 1:3], src_ap[:, 1:3])
                    else:
                        nc.sync.dma_start(dst, src_ap)
                    st[s]["xg"].append(xt)
                    for i in range(3):
                        st[s]["x"].append(xt[:, i * T:(i + 1) * T])

            def phaseA_moments(s, c, sq="A"):
                """sum(x) on DVE (4x) + sum(x^2) on ACT (Square+accum), DVE
                (x*x then accum), or Pool (x*x) + DVE accum."""
                d = st[s]
                if c == 0:
                    d["Mx"] = spool.tile([128, NCH], f32, name="Mx", tag="Mx")
                    d["Mx2"] = spool.tile([128, NCH], f32, name="Mx2", tag="Mx2")
                xt = d["x"][c]
                j0 = jpool.tile([128, T], bf16, name="junk", tag="junk")
                if FRONT_MX[c] == "A":
                    nc.scalar.activation(j0[:], xt, Act.Identity,
                                         accum_out=d["Mx"][:, c:c + 1])
                else:
                    nc.vector.tensor_scalar(j0[:], xt, 0.0, 0.0, Alu.add,
                                            Alu.add,
                                            accum_out=d["Mx"][:, c:c + 1])
                if sq == "A":
                    j1 = jpool.tile([128, T], bf16, name="junk", tag="junk")
                    nc.scalar.activation(j1[:], xt, Act.Square,
                                         accum_out=d["Mx2"][:, c:c + 1])
                else:
                    x2 = jpool.tile([128, T], bf16, name="junk", tag="junk")
                    if sq == "P":
                        nc.gpsimd.tensor_tensor(x2[:], xt, xt, Alu.mult)
                    else:
                        nc.vector.tensor_tensor(x2[:], xt, xt, Alu.mult)
                    j1 = jpool.tile([128, T], bf16, name="junk", tag="junk")
                    nc.vector.tensor_scalar(j1[:], x2[:], 0.0, 0.0, Alu.add,
                                            Alu.add,
                                            accum_out=d["Mx2"][:, c:c + 1])

            def phaseA_mm1(s, c):
                d = st[s]
                if c == 0:
                    d["ph1"] = ph1p.tile([A, T], f32, name="ph1", tag="ph1")
                xt = d["x"][c]
                for lo, hi in HALVES:
                    nc.tensor.matmul(d["ph1"][:, lo:hi], w1xT_t[c],
                                     xt[:, lo:hi], start=(c == 0),
                                     stop=(c == NCH - 1), skip_group_check=True)

            def newton_rsqrt(v_ap, out_ap, n, iters):
                """out = 1/sqrt(v) elementwise on a [128, n] fp32 AP.
                Seed r0 = 2/(1+v) (robust for any v>0), then Newton
                iterations r' = r*(1.5 - 0.5*v*r^2), 3 DVE ops each:
                  t = r*r;  u = (t*-0.5)*v;  r' = (u+1.5)*r."""
                t0 = smpool.tile([128, n], f32, name="nw0", tag="nw0")
                t1 = smpool.tile([128, n], f32, name="nw1", tag="nw1")
                r = smpool.tile([128, n], f32, name="nwr", tag="nwr")
                nc.vector.tensor_scalar(t0[:], v_ap, 0.5, 0.5, Alu.mult, Alu.add)
                nc.vector.reciprocal(r[:], t0[:])
                for it in range(iters):
                    dst = out_ap if it == iters - 1 else r[:]
                    nc.vector.tensor_tensor(t0[:], r[:], r[:], Alu.mult)
                    nc.vector.scalar_tensor_tensor(t1[:], t0[:], -0.5, v_ap,
                                                   Alu.mult, Alu.mult)
                    nc.vector.scalar_tensor_tensor(dst, t1[:], 1.5, r[:],
                                                   Alu.add, Alu.mult)

            def phaseB_stats(s):
                """mean/std from Mx/Mx2 + mean-half of the bias matvec."""
                d = st[s]
                meanc = smpool.tile([128, NCH], f32, name="meanc", tag="meanc")
                nc.vector.tensor_scalar(meanc[:], d["Mx"][:], 1.0 / T, None, Alu.mult)
                pmv = pmvp.tile([A, 1], f32, name="pmv", tag="pmv")
                d["pmv"] = pmv
                for k in range(NCH):
                    nc.tensor.matmul(pmv[:], wms_t[k], meanc[:, k:k + 1],
                                     start=(k == 0), stop=False,
                                     skip_group_check=True)
                # unbiased var = (Mx2 - T*mean^2) / (T-1);  T*mean^2 = mean*Mx
                tm2 = smpool.tile([128, NCH], f32, name="tm2", tag="tm2")
                nc.vector.tensor_tensor(tm2[:], meanc[:], d["Mx"][:], Alu.mult)
                vdiff = smpool.tile([128, NCH], f32, name="vdiff", tag="vdiff")
                nc.vector.scalar_tensor_tensor(vdiff[:], tm2[:], -1.0, d["Mx2"][:],
                                               Alu.mult, Alu.add)
                v = smpool.tile([128, NCH], f32, name="v", tag="v")
                nc.vector.tensor_scalar(v[:], vdiff[:], 1.0 / (T - 1.0), CLAMP,
                                        Alu.mult, Alu.max)
                std_t = smpool.tile([128, NCH], f32, name="std_t", tag="std_t")
                d["std_t"] = std_t
                rs = smpool.tile([128, NCH], f32, name="rs", tag="rs")
                newton_rsqrt(v[:], rs[:], NCH, 2)
                nc.vector.tensor_tensor(std_t[:], v[:], rs[:], Alu.mult)

            def phaseB_matvec2(s):
                """std-half of the matvec + btot."""
                d = st[s]
                pmv, std_t = d["pmv"], d["std_t"]
                for k in range(NCH):
                    nc.tensor.matmul(pmv[:], wms_t[NCH + k], std_t[:, k:k + 1],
                                     start=False, stop=(k == NCH - 1),
                                     skip_group_check=True)
                btot = smpool.tile([A, 1], f32, name="btot", tag="btot")
                nc.vector.tensor_tensor(btot[:], pmv[:], b1_t[:], Alu.add)
                d["btot"] = btot

            def phaseB_relu(s):
                d = st[s]
                rt = rpool.tile([A, T], bf16, name="r", tag="r")
                nc.scalar.activation(rt[:], d["ph1"][:], Act.Relu, bias=d["btot"][:])
                d["rt"] = rt

            def phaseB_tanh(s):
                d = st[s]
                gt = gpool.tile([A, T], bf16, name="g", tag="g")
                nc.scalar.activation(gt[:], d["rt"][:], Act.Tanh, bias=add1_t[:],
                                     scale=inv1_t[:])
                d["g"] = gt

            def phaseB(s):
                phaseB_stats(s)
                phaseB_matvec2(s)
                phaseB_relu(s)
                phaseB_tanh(s)

            def phaseC_mm2exp(s, c):
                """PE matmul2 + ACT exp for chunk c (emitted one chunk ahead
                of the DVE consumer so the in-order ACT stream never starves
                the DVE)."""
                d = st[s]
                if c == 0:
                    d["S0"] = spool.tile([128, NCH], f32, name="S0", tag="S0")
                    d["S1"] = spool.tile([128, NCH], f32, name="S1", tag="S1")
                    d["S2"] = spool.tile([128, NCH], f32, name="S2", tag="S2")
                    d["E"] = [None] * NCH
                p2 = p2p.tile([128, T], f32, name="p2", tag="p2")
                wsl = w2T_t[:, c * 128:(c + 1) * 128]
                for lo, hi in HALVES:
                    nc.tensor.matmul(p2[:, lo:hi], wsl, d["g"][:, lo:hi],
                                     start=True, stop=True)
                E = epool.tile([128, T], bf16, name="E", tag="E")
                nc.scalar.activation(E[:], p2[:], Act.Exp,
                                     bias=b2p_t[:, c:c + 1], scale=inv2_t[:, c:c + 1])
                d["E"][c] = E

            def phaseC_dve(s, c, qt_pool=True, s1_act=False):
                """eb/pt/S1 for chunk c; qt on Pool (or DVE); S2 of chunk c-2
                (deferred two chunks so the DVE never waits on Pool's qt)."""
                d = st[s]
                E = d["E"][c]
                d["E"][c] = None
                eb = ebpool.tile([128, T], bf16, name="eb", tag="eb")
                nc.vector.tensor_scalar(eb[:], E[:], 1.0, 0.0, Alu.max, Alu.add,
                                        accum_out=d["S0"][:, c:c + 1])
                xt = d["x"][c]
                pt = ppool.tile([128, T], bf16, name="p", tag="p")
                nc.vector.tensor_tensor(pt[:], eb[:], xt, Alu.mult)
                qt = qpool.tile([128, T], bf16, name="q", tag="q")
                if qt_pool:
                    nc.gpsimd.tensor_tensor(qt[:], pt[:], xt, Alu.mult)
                else:
                    nc.vector.tensor_tensor(qt[:], pt[:], xt, Alu.mult)
                d["qt_%d" % c] = qt
                if s1_act:
                    j1 = jpool.tile([128, T], bf16, name="junk", tag="junk")
                    nc.scalar.activation(j1[:], pt[:], Act.Identity,
                                         accum_out=d["S1"][:, c:c + 1])
                else:
                    j1 = jpool.tile([128, T], bf16, name="junk", tag="junk")
                    nc.vector.tensor_scalar(j1[:], pt[:], 0.0, 0.0, Alu.add,
                                            Alu.add,
                                            accum_out=d["S1"][:, c:c + 1])

            def phaseC_s2(s, c):
                """S2 accumulate for chunk c (reads qt produced on Pool)."""
                d = st[s]
                qt = d.pop("qt_%d" % c)
                j2 = jpool.tile([128, T], bf16, name="junk", tag="junk")
                nc.vector.tensor_scalar(j2[:], qt[:], 0.0, 0.0, Alu.add, Alu.add,
                                        accum_out=d["S2"][:, c:c + 1])

            def store_half(s, half, srct):
                ptr = ptrp.tile([NCH, 128], f32, name="ptr", tag="ptr")
                nc.tensor.transpose(ptr[:], srct[:], ident_t[:])
                ost = opool.tile([NCH, 128], f32, name="ost", tag="ost")
                nc.scalar.copy(ost[:], ptr[:])
                dst = out[s, half * C:(half + 1) * C]
                dst = dst.rearrange("(ci p) -> ci p", p=128)
                nc.sync.dma_start(dst, ost[:])

            def sample_out_mu(s):
                """mu (needs only S0/S1) + transpose (PE) + store."""
                d = st[s]
                rc = smpool.tile([128, NCH], f32, name="rc", tag="rc")
                nc.vector.reciprocal(rc[:], d["S0"][:])
                d["rc"] = rc
                mu = opool.tile([128, NCH], f32, name="mu", tag="mu")
                nc.vector.tensor_tensor(mu[:], d["S1"][:], rc[:], Alu.mult)
                d["mu"] = mu
                store_half(s, 0, mu)

            def sample_out_sg(s):
                """sg (needs S2) + transpose (PE) + store."""
                d = st[s]
                rc, mu = d["rc"], d["mu"]
                sg = opool.tile([128, NCH], f32, name="sg", tag="sg")
                ex2 = smpool.tile([128, NCH], f32, name="ex2", tag="ex2")
                nc.vector.tensor_tensor(ex2[:], d["S2"][:], rc[:], Alu.mult)
                mu2 = smpool.tile([128, NCH], f32, name="mu2", tag="mu2")
                nc.vector.tensor_tensor(mu2[:], mu[:], mu[:], Alu.mult)
                sg2 = smpool.tile([128, NCH], f32, name="sg2", tag="sg2")
                nc.vector.scalar_tensor_tensor(sg2[:], mu2[:], -1.0, ex2[:],
                                               Alu.mult, Alu.add)
                v2 = smpool.tile([128, NCH], f32, name="v2", tag="v2")
                nc.vector.tensor_scalar(v2[:], sg2[:], 1.0, CLAMP, Alu.mult, Alu.max)
                rsg = smpool.tile([128, NCH], f32, name="rsg", tag="rsg")
                newton_rsqrt(v2[:], rsg[:], NCH, 3)
                nc.vector.tensor_tensor(sg[:], v2[:], rsg[:], Alu.mult)
                store_half(s, 1, sg)

            # ---------------- constant loads (interleaved with x below) ----
            def load_w1xT():
                t = cpool.tile([128, NCH * A], bf16, name="w1xall", tag="w1xall")
                src_ap = w1xT.rearrange("(c p) a -> p c a", p=128)
                nc.sync.dma_start(t[:].rearrange("p (c a) -> p c a", a=A), src_ap)
                return [t[:, c * A:(c + 1) * A] for c in range(NCH)]

            def load_params():
                global b1_t, inv1_t, add1_t, inv2_t, b2p_t, w2T_t, wms_t, ident_t
                b1_t = cpool.tile([A, 1], f32, name="b1", tag="b1")
                nc.sync.dma_start(b1_t[:], b1d[:])
                inv1_t = cpool.tile([A, 1], f32, name="inv1", tag="inv1")
                nc.sync.dma_start(inv1_t[:], inv1d[:])
                add1_t = cpool.tile([A, 1], f32, name="add1", tag="add1")
                nc.sync.dma_start(add1_t[:], add1d[:])
                inv2_t = cpool.tile([128, NCH], f32, name="inv2", tag="inv2")
                nc.sync.dma_start(inv2_t[:], inv2d[:])
                b2p_t = cpool.tile([128, NCH], f32, name="b2p", tag="b2p")
                nc.sync.dma_start(b2p_t[:], b2pd[:])
                w2T_t = cpool.tile([A, C], bf16, name="w2T", tag="w2T")
                nc.sync.dma_start(w2T_t[:], w2T[:])
                ident_t = cpool.tile([128, 128], f32, name="ident", tag="ident")
                nc.sync.dma_start(ident_t[:], identd[:])
                wt = cpool.tile([128, 2 * NCH * A], f32, name="wmsall", tag="wmsall")
                src_ap = wmsT.rearrange("(k p) a -> p k a", p=128)
                nc.sync.dma_start(wt[:].rearrange("p (k a) -> p k a", a=A), src_ap)
                wms_t = [wt[:, k * A:(k + 1) * A] for k in range(2 * NCH)]

            def body():
                global w1xT_t
                # prologue: phase A of samples 0/1, weights interleaved,
                # sample 2's DMA prefetched.  sum(x^2) rotates over
                # DVE/ACT/Pool so no engine serializes the (un-overlapped)
                # prologue.
                dma_x(0, groups=[0], split_first=True)
                w1xT_t = load_w1xT()
                dma_x(0, groups=[1, 2, 3])
                for c in range(NCH):
                    phaseA_moments(0, c, sq=PRO_SQ[c])
                    phaseA_mm1(0, c)
                load_params()
                dma_x(1)
                for c in range(NCH):
                    phaseA_moments(1, c, sq=PRO_SQ[(c + 6) % NCH])
                dma_x(2)
                phaseB(0)
                for c in range(NCH):
                    phaseA_mm1(1, c)
                # steady state: C(s) carries A(s+2) moments, B(s+1) spread
                # over c3/c4/c5/c8, and A(s+2)'s matmul1 in the c>=8 shadow
                # of relu(s+1) freeing the ph1 slot.  ACT's exp runs one
                # chunk ahead of the DVE; S2 accumulation runs one chunk
                # behind (its qt comes from the Pool engine).
                # (s, c) pairs for the exp-lookahead stream, two chunks ahead
                # of the DVE consumer.  g(s) is ready by c==8 of C(s-1), so
                # (s+1, 0) may be emitted from c==10 of C(s) onward.
                mm2exp_seq = [(s, c) for s in range(SPC) for c in range(NCH)]
                mm2exp_pos = 0

                def emit_mm2exp_upto(i):
                    nonlocal mm2exp_pos
                    while mm2exp_pos <= i and mm2exp_pos < len(mm2exp_seq):
                        phaseC_mm2exp(*mm2exp_seq[mm2exp_pos])
                        mm2exp_pos += 1

                emit_mm2exp_upto(1)
                for s in range(SPC):
                    has_a = s + 2 < SPC          # phase-A work interleaved?
                    qt_map = FRONT_QT if has_a else TAIL_QT
                    if s == SPC - 1:
                        qt_map = qt_map[:NCH - 2] + "DD"
                    s1_map = FRONT_S1 if has_a else TAIL_S1
                    for c in range(NCH):
                        emit_mm2exp_upto(s * NCH + c + 2)
                        phaseC_dve(s, c, qt_pool=(qt_map[c] == "P"),
                                   s1_act=(s1_map[c] == "A"))
                        if c > 1:
                            phaseC_s2(s, c - 2)
                        # deferred flush of the previous sample: its last S2s
                        # and the whole output stage run here, where the
                        # Pool's qt and the serial small-op chain can hide
                        # under this sample's chunk stream.
                        if c == 0 and s - 1 in st:
                            phaseC_s2(s - 1, NCH - 2)
                            sample_out_mu(s - 1)
                        if c == 1 and s - 1 in st:
                            phaseC_s2(s - 1, NCH - 1)
                        if c == 2 and s - 1 in st:
                            sample_out_sg(s - 1)
                            del st[s - 1]
                        if s + 3 < SPC and c == 0:
                            dma_x(s + 3)
                        if has_a:
                            if s == 0:
                                # x(2)'s DMA was only issued in the prologue:
                                # shift A(2) two chunks later so the in-order
                                # DVE stream doesn't park on it.
                                if c >= 2:
                                    phaseA_moments(2, c - 2, sq=FRONT_SQ[c - 2])
                            else:
                                phaseA_moments(s + 2, c, sq=FRONT_SQ[c])
                                if s == 1 and c < 2:
                                    phaseA_moments(2, 10 + c, sq=FRONT_SQ[10 + c])
                        bc = (3, 4, 5, 8)
                        if c == bc[0] and s + 1 < SPC:
                            phaseB_stats(s + 1)
                        if c == bc[1] and s + 1 < SPC:
                            phaseB_matvec2(s + 1)
                        if c == bc[2] and s + 1 < SPC:
                            phaseB_relu(s + 1)
                        if c == bc[3] and s + 1 < SPC:
                            phaseB_tanh(s + 1)
                        # mm1(s+2) reuses the single ph1 bank; its first
                        # matmul must be emitted after relu(s+1) has read it.
                        if has_a and c >= 8:
                            for cc in range(3 * (c - 8), 3 * (c - 8) + 3):
                                phaseA_mm1(s + 2, cc)
                s = SPC - 1
                phaseC_s2(s, NCH - 2)
                sample_out_mu(s)
                phaseC_s2(s, NCH - 1)
                sample_out_sg(s)
                del st[s]

            if loop_reps == 1:
                body()
            else:
                with tc.For_i(0, loop_reps, 1):
                    body()

    nc.compile()
    return nc


def _get_module(loop_reps=1):
    key = loop_reps
    if key not in _CACHE:
        _CACHE[key] = _build_module(loop_reps)
    return _CACHE[key]


def _host_prep(inputs):
    """Precompute folded parameters and shard inputs. Returns per-core in_maps."""
    x = np.asarray(inputs["x"])
    W1 = np.asarray(inputs["W1"], np.float32)
    b1 = np.asarray(inputs["b1"], np.float32)
    g1 = np.asarray(inputs["g1"], np.float32)
    beta1 = np.asarray(inputs["beta1"], np.float32)
    rm1 = np.asarray(inputs["rm1"], np.float32)
    rv1 = np.asarray(inputs["rv1"], np.float32)
    W2 = np.asarray(inputs["W2"], np.float32)
    b2 = np.asarray(inputs["b2"], np.float32)
    g2 = np.asarray(inputs["g2"], np.float32)
    rv2 = np.asarray(inputs["rv2"], np.float32)

    inv1 = (g1 / np.sqrt(rv1 + BN_EPS)).astype(np.float32)
    add1 = (beta1 - rm1 * inv1).astype(np.float32)
    inv2 = (g2 / np.sqrt(rv2 + BN_EPS)).astype(np.float32)
    b2p = (inv2 * b2).astype(np.float32)

    const = {
        "w1xT": np.ascontiguousarray(W1[:, :C].T).astype(ml_dtypes.bfloat16),
        "wmsT": np.ascontiguousarray(W1[:, C:].T).astype(np.float32),
        "w2T": np.ascontiguousarray(W2.T).astype(ml_dtypes.bfloat16),
        "b1d": b1.reshape(A, 1),
        "inv1d": inv1.reshape(A, 1),
        "add1d": add1.reshape(A, 1),
        "inv2d": np.ascontiguousarray(inv2.reshape(NCH, 128).T),
        "b2pd": np.ascontiguousarray(b2p.reshape(NCH, 128).T),
        "identd": np.eye(128, dtype=np.float32),
    }
    xbf = x.astype(ml_dtypes.bfloat16)
    in_maps = []
    for core in range(N_CORES):
        m = dict(const)
        m["xbf"] = np.ascontiguousarray(xbf[core * SPC:(core + 1) * SPC])
        in_maps.append(m)
    return in_maps


def kernel(**inputs):
    from concourse.bass_utils import run_bass_kernel_spmd

    nc = _get_module(loop_reps=1)
    in_maps = _host_prep(inputs)
    res = run_bass_kernel_spmd(nc, in_maps, core_ids=list(range(N_CORES)))
    out = np.concatenate([res.results[i]["out"] for i in range(N_CORES)], axis=0)
    return out.astype(np.float32)


# revision 39
# speedup vs baseline: 1241.7297x; 1.0201x over previous
"""AttentiveStatPooling Trainium2 kernel (8-core SPMD, data-parallel over batch).

Contract: kernel(**inputs) takes the FULL unsharded inputs (as produced by
reference.setup_inputs()) and returns the FULL [B, 2C] output.

Math (per sample, identical to the jax reference):
  mean/std over T of x;  h = relu(Wx@x + (Wm@mean + Ws@std + b1));
  g = tanh(BN1(h));  l = BN2scale * relu(W2@g + b2)  (the BN2 shift cancels in
  the softmax and is dropped);  w = softmax(l, axis=T);
  out = [sum(x*w), sqrt(clip(sum(x^2*w) - mu^2, 1e-4))].

Implementation notes (v3 — four-engine balance):
  - batch 32 split 4 samples/core across 8 NeuronCores (pure DP).
  - x shipped in bf16 (halves DMA; all elementwise work runs in bf16 so the
    DVE hits its 2x (tensor_tensor) / 4x (tensor_scalar) perf modes).
  - per-chunk softmax stats with fused accumulates (no standalone reduce):
      eb = max(E,1)  tensor_scalar(max) 4x, fused accum -> S0   [DVE]
      pt = eb*x      tensor_tensor 2x                           [DVE]
      S1 = sum(pt)   tensor_scalar(id) 4x + accum               [DVE or ACT]
      qt = pt*x      tensor_tensor                              [Pool or DVE]
      S2 = sum(qt)   tensor_scalar(id) 4x + accum               [DVE]
    The qt multiply rides the otherwise-idle GPSIMD/Pool engine (plain
    tensor_tensor is the only elementwise op walrus allows there); its S2
    accumulate is deferred two chunks so the DVE never waits on the Pool.
  - x moments per chunk: sum(x) on DVE (tensor_scalar 4x + accum),
    sum(x^2) on ACT (Square+accum) / DVE (x*x + accum) per the *_SQ maps.
    All ACT functions used (Exp/Tanh/Square/Relu/Identity) live in one
    activation table set, so there are no table reloads.
  - softmax needs no max-subtraction (logits bounded, per-row shift cancels);
    relu inside the softmax realized as max(exp(l), 1).
  - sqrt via Newton/rsqrt on the vector engine (avoids ACT table switches),
    3 DVE ops per iteration via scalar_tensor_tensor.
  - emission is software-pipelined: mm2+exp run two chunks ahead of the DVE
    consumer, phase A of sample s+2 and phase B of sample s+1 interleave
    into phase C of sample s, and the previous sample's S2 flush + output
    stage defer into the next sample's chunk stream.  The FRONT_*/TAIL_*/PRO_* knob
    strings balance per-chunk work across DVE/ACT/Pool for the "front"
    samples (which carry phase-A work) and the "tail" samples (which don't).
"""

import numpy as np
import ml_dtypes

B, C, T, A = 32, 1536, 1000, 128
N_CORES = 8
SPC = B // N_CORES        # samples per core
NCH = C // 128            # 12 channel chunks of 128
BN_EPS = 1e-5
CLAMP = 1e-4
HALVES = ((0, 512), (512, 1000))   # psum-bank-aligned split of T

_CACHE = {}

# Engine-balance knobs (index = c % 12).  For each chunk of "front" samples
# (those that also carry phase-A work of sample s+2) and "tail" samples:
#   qt engine: 'P' (Pool) or 'D' (DVE)
#   S1 accumulate: 'D' (DVE tensor_scalar) or 'A' (ACT identity+accum)
#   phase-A sum(x^2): 'A' (ACT Square), 'D' (DVE x*x+accum), 'P' (Pool x*x
#   + DVE accum)
FRONT_QT = "PPPPPPPPPPPP"
FRONT_S1 = "DDDDDDDDDDDD"
FRONT_SQ = "AAADAAADAAAD"
TAIL_QT = "DPPPDPPPDPPP"
TAIL_S1 = "ADADDADADDAD"
PRO_SQ = "DADADADADADA"
FRONT_MX = "DDDDDDDDDDDD"   # prologue samples 0/1 (rotated by 6 for s=1)


def _build_module(loop_reps=1):
    import concourse.tile as tile
    from concourse import bacc, mybir
    from contextlib import ExitStack

    f32, bf16 = mybir.dt.float32, mybir.dt.bfloat16
    Alu = mybir.AluOpType
    Act = mybir.ActivationFunctionType

    nc = bacc.Bacc("TRN2", target_bir_lowering=False, debug=False,
                   num_devices=N_CORES)

    xbf = nc.dram_tensor("xbf", [SPC, C, T], bf16, kind="ExternalInput").ap()
    w1xT = nc.dram_tensor("w1xT", [C, A], bf16, kind="ExternalInput").ap()
    wmsT = nc.dram_tensor("wmsT", [2 * C, A], f32, kind="ExternalInput").ap()
    w2T = nc.dram_tensor("w2T", [A, C], bf16, kind="ExternalInput").ap()
    b1d = nc.dram_tensor("b1d", [A, 1], f32, kind="ExternalInput").ap()
    inv1d = nc.dram_tensor("inv1d", [A, 1], f32, kind="ExternalInput").ap()
    add1d = nc.dram_tensor("add1d", [A, 1], f32, kind="ExternalInput").ap()
    inv2d = nc.dram_tensor("inv2d", [128, NCH], f32, kind="ExternalInput").ap()
    b2pd = nc.dram_tensor("b2pd", [128, NCH], f32, kind="ExternalInput").ap()
    identd = nc.dram_tensor("identd", [128, 128], f32, kind="ExternalInput").ap()
    out = nc.dram_tensor("out", [SPC, 2 * C], f32, kind="ExternalOutput").ap()

    with tile.TileContext(nc) as tc:
        with ExitStack() as ctx:
            cpool = ctx.enter_context(tc.tile_pool(name="const", bufs=1))
            xpool = ctx.enter_context(tc.tile_pool(name="x", bufs=14))
            epool = ctx.enter_context(tc.tile_pool(name="e", bufs=3))
            ebpool = ctx.enter_context(tc.tile_pool(name="eb", bufs=3))
            ppool = ctx.enter_context(tc.tile_pool(name="p", bufs=3))
            qpool = ctx.enter_context(tc.tile_pool(name="q", bufs=4))
            jpool = ctx.enter_context(tc.tile_pool(name="junk", bufs=6))
            rpool = ctx.enter_context(tc.tile_pool(name="r", bufs=2))
            gpool = ctx.enter_context(tc.tile_pool(name="g", bufs=2))
            spool = ctx.enter_context(tc.tile_pool(name="stats", bufs=4))
            smpool = ctx.enter_context(tc.tile_pool(name="small", bufs=8))
            opool = ctx.enter_context(tc.tile_pool(name="ostage", bufs=4))
            ph1p = ctx.enter_context(tc.tile_pool(name="ph1", bufs=1, space="PSUM"))
            p2p = ctx.enter_context(tc.tile_pool(name="p2", bufs=2, space="PSUM"))
            pmvp = ctx.enter_context(tc.tile_pool(name="pmv", bufs=1, space="PSUM"))
            ptrp = ctx.enter_context(tc.tile_pool(name="ptr", bufs=1, space="PSUM"))

            st = {}   # per-sample state

            def dma_x(s, groups=range(4), split_first=False):
                if s not in st:
                    st[s] = {"xg": [], "x": []}
                for g in groups:
                    xt = xpool.tile([128, 3 * T], bf16, name="x", tag="x")
                    src_ap = xbf[s, g * 384:(g + 1) * 384, :]
                    src_ap = src_ap.rearrange("(c p) t -> p c t", p=128)
                    dst = xt[:].rearrange("p (c t) -> p c t", t=T)
                    if split_first and g == 0:
                        # land chunk 0 first so phase A can start sooner
                        nc.sync.dma_start(dst[:, 0:1], src_ap[:, 0:1])
                        nc.sync.dma_start(dst[:, 1:3], src_ap[:, 1:3])
                    else:
                        nc.sync.dma_start(dst, src_ap)
                    st[s]["xg"].append(xt)
                    for i in range(3):
                        st[s]["x"].append(xt[:, i * T:(i + 1) * T])

            def phaseA_moments(s, c, sq="A"):
                """sum(x) on DVE (4x) + sum(x^2) on ACT (Square+accum), DVE
                (x*x then accum), or Pool (x*x) + DVE accum."""
                d = st[s]
                if c == 0:
                    d["Mx"] = spool.tile([128, NCH], f32, name="Mx", tag="Mx")
                    d["Mx2"] = spool.tile([128, NCH], f32, name="Mx2", tag="Mx2")
                xt = d["x"][c]
                j0 = jpool.tile([128, T], bf16, name="junk", tag="junk")
                if FRONT_MX[c] == "A":
                    nc.scalar.activation(j0[:], xt, Act.Identity,
                                         accum_out=d["Mx"][:, c:c + 1])
                else:
                    nc.vector.tensor_scalar(j0[:], xt, 0.0, 0.0, Alu.add,
                                            Alu.add,
                                            accum_out=d["Mx"][:, c:c + 1])
                if sq == "A":
                    j1 = jpool.tile([128, T], bf16, name="junk", tag="junk")
                    nc.scalar.activation(j1[:], xt, Act.Square,
                                         accum_out=d["Mx2"][:, c:c + 1])
                else:
                    x2 = jpool.tile([128, T], bf16, name="junk", tag="junk")
                    if sq == "P":
                        nc.gpsimd.tensor_tensor(x2[:], xt, xt, Alu.mult)
                    else:
                        nc.vector.tensor_tensor(x2[:], xt, xt, Alu.mult)
                    j1 = jpool.tile([128, T], bf16, name="junk", tag="junk")
                    nc.vector.tensor_scalar(j1[:], x2[:], 0.0, 0.0, Alu.add,
                                            Alu.add,
                                            accum_out=d["Mx2"][:, c:c + 1])

            def phaseA_mm1(s, c):
                d = st[s]
                if c == 0:
                    d["ph1"] = ph1p.tile([A, T], f32, name="ph1", tag="ph1")
                xt = d["x"][c]
                for lo, hi in HALVES:
                    nc.tensor.matmul(d["ph1"][:, lo:hi], w1xT_t[c],
                                     xt[:, lo:hi], start=(c == 0),
                                     stop=(c == NCH - 1), skip_group_check=True)

            def newton_rsqrt(v_ap, out_ap, n, iters):
                """out = 1/sqrt(v) elementwise on a [128, n] fp32 AP.
                Seed r0 = 2/(1+v) (robust for any v>0), then Newton
                iterations r' = r*(1.5 - 0.5*v*r^2), 3 DVE ops each:
                  t = r*r;  u = (t*-0.5)*v;  r' = (u+1.5)*r."""
                t0 = smpool.tile([128, n], f32, name="nw0", tag="nw0")
                t1 = smpool.tile([128, n], f32, name="nw1", tag="nw1")
                r = smpool.tile([128, n], f32, name="nwr", tag="nwr")
                nc.vector.tensor_scalar(t0[:], v_ap, 0.5, 0.5, Alu.mult, Alu.add)
                nc.vector.reciprocal(r[:], t0[:])
                for it in range(iters):
                    dst = out_ap if it == iters - 1 else r[:]
                    nc.vector.tensor_tensor(t0[:], r[:], r[:], Alu.mult)
                    nc.vector.scalar_tensor_tensor(t1[:], t0[:], -0.5, v_ap,
                                                   Alu.mult, Alu.mult)
                    nc.vector.scalar_tensor_tensor(dst, t1[:], 1.5, r[:],
                                                   Alu.add, Alu.mult)

            def phaseB_stats(s):
                """mean/std from Mx/Mx2 + mean-half of the bias matvec."""
                d = st[s]
                meanc = smpool.tile([128, NCH], f32, name="meanc", tag="meanc")
                nc.vector.tensor_scalar(meanc[:], d["Mx"][:], 1.0 / T, None, Alu.mult)
                pmv = pmvp.tile([A, 1], f32, name="pmv", tag="pmv")
                d["pmv"] = pmv
                for k in range(NCH):
                    nc.tensor.matmul(pmv[:], wms_t[k], meanc[:, k:k + 1],
                                     start=(k == 0), stop=False,
                                     skip_group_check=True)
                # unbiased var = (Mx2 - T*mean^2) / (T-1);  T*mean^2 = mean*Mx
                tm2 = smpool.tile([128, NCH], f32, name="tm2", tag="tm2")
                nc.vector.tensor_tensor(tm2[:], meanc[:], d["Mx"][:], Alu.mult)
                vdiff = smpool.tile([128, NCH], f32, name="vdiff", tag="vdiff")
                nc.vector.scalar_tensor_tensor(vdiff[:], tm2[:], -1.0, d["Mx2"][:],
                                               Alu.mult, Alu.add)
                v = smpool.tile([128, NCH], f32, name="v", tag="v")
                nc.vector.tensor_scalar(v[:], vdiff[:], 1.0 / (T - 1.0), CLAMP,
                                        Alu.mult, Alu.max)
                std_t = smpool.tile([128, NCH], f32, name="std_t", tag="std_t")
                d["std_t"] = std_t
                rs = smpool.tile([128, NCH], f32, name="rs", tag="rs")
                newton_rsqrt(v[:], rs[:], NCH, 2)
                nc.vector.tensor_tensor(std_t[:], v[:], rs[:], Alu.mult)

            def phaseB_matvec2(s):
                """std-half of the matvec + btot."""
                d = st[s]
                pmv, std_t = d["pmv"], d["std_t"]
                for k in range(NCH):
                    nc.tensor.matmul(pmv[:], wms_t[NCH + k], std_t[:, k:k + 1],
                                     start=False, stop=(k == NCH - 1),
                                     skip_group_check=True)
                btot = smpool.tile([A, 1], f32, name="btot", tag="btot")
                nc.vector.tensor_tensor(btot[:], pmv[:], b1_t[:], Alu.add)
                d["btot"] = btot

            def phaseB_relu(s):
                d = st[s]
                rt = rpool.tile([A, T], bf16, name="r", tag="r")
                nc.scalar.activation(rt[:], d["ph1"][:], Act.Relu, bias=d["btot"][:])
                d["rt"] = rt

            def phaseB_tanh(s):
                d = st[s]
                gt = gpool.tile([A, T], bf16, name="g", tag="g")
                nc.scalar.activation(gt[:], d["rt"][:], Act.Tanh, bias=add1_t[:],
                                     scale=inv1_t[:])
                d["g"] = gt

            def phaseB(s):
                phaseB_stats(s)
                phaseB_matvec2(s)
                phaseB_relu(s)
                phaseB_tanh(s)

            def phaseC_mm2exp(s, c):
                """PE matmul2 + ACT exp for chunk c (emitted one chunk ahead
                of the DVE consumer so the in-order ACT stream never starves
                the DVE)."""
                d = st[s]
                if c == 0:
                    d["S0"] = spool.tile([128, NCH], f32, name="S0", tag="S0")
                    d["S1"] = spool.tile([128, NCH], f32, name="S1", tag="S1")
                    d["S2"] = spool.tile([128, NCH], f32, name="S2", tag="S2")
                    d["E"] = [None] * NCH
                p2 = p2p.tile([128, T], f32, name="p2", tag="p2")
                wsl = w2T_t[:, c * 128:(c + 1) * 128]
                for lo, hi in HALVES:
                    nc.tensor.matmul(p2[:, lo:hi], wsl, d["g"][:, lo:hi],
                                     start=True, stop=True)
                E = epool.tile([128, T], bf16, name="E", tag="E")
                nc.scalar.activation(E[:], p2[:], Act.Exp,
                                     bias=b2p_t[:, c:c + 1], scale=inv2_t[:, c:c + 1])
                d["E"][c] = E

            def phaseC_dve(s, c, qt_pool=True, s1_act=False):
                """eb/pt/S1 for chunk c; qt on Pool (or DVE); S2 of chunk c-2
                (deferred two chunks so the DVE never waits on Pool's qt)."""
                d = st[s]
                E = d["E"][c]
                d["E"][c] = None
                eb = ebpool.tile([128, T], bf16, name="eb", tag="eb")
                nc.vector.tensor_scalar(eb[:], E[:], 1.0, 0.0, Alu.max, Alu.add,
                                        accum_out=d["S0"][:, c:c + 1])
                xt = d["x"][c]
                pt = ppool.tile([128, T], bf16, name="p", tag="p")
                nc.vector.tensor_tensor(pt[:], eb[:], xt, Alu.mult)
                qt = qpool.tile([128, T], bf16, name="q", tag="q")
                if qt_pool:
                    nc.gpsimd.tensor_tensor(qt[:], pt[:], xt, Alu.mult)
                else:
                    nc.vector.tensor_tensor(qt[:], pt[:], xt, Alu.mult)
                d["qt_%d" % c] = qt
                if s1_act:
                    j1 = jpool.tile([128, T], bf16, name="junk", tag="junk")
                    nc.scalar.activation(j1[:], pt[:], Act.Identity,
                                         accum_out=d["S1"][:, c:c + 1])
                else:
                    j1 = jpool.tile([128, T], bf16, name="junk", tag="junk")
                    nc.vector.tensor_scalar(j1[:], pt[:], 0.0, 0.0, Alu.add,
                                            Alu.add,
                                            accum_out=d["S1"][:, c:c + 1])

            def phaseC_s2(s, c):
                """S2 accumulate for chunk c (reads qt produced on Pool)."""
                d = st[s]
                qt = d.pop("qt_%d" % c)
                j2 = jpool.tile([128, T], bf16, name="junk", tag="junk")
                nc.vector.tensor_scalar(j2[:], qt[:], 0.0, 0.0, Alu.add, Alu.add,
                                        accum_out=d["S2"][:, c:c + 1])

            def store_half(s, half, srct):
                ptr = ptrp.tile([NCH, 128], f32, name="ptr", tag="ptr")
                nc.tensor.transpose(ptr[:], srct[:], ident_t[:])
                ost = opool.tile([NCH, 128], f32, name="ost", tag="ost")
                nc.scalar.copy(ost[:], ptr[:])
                dst = out[s, half * C:(half + 1) * C]
                dst = dst.rearrange("(ci p) -> ci p", p=128)
                nc.sync.dma_start(dst, ost[:])

            def sample_out_mu(s):
                """mu (needs only S0/S1) + transpose (PE) + store."""
                d = st[s]
                rc = smpool.tile([128, NCH], f32, name="rc", tag="rc")
                nc.vector.reciprocal(rc[:], d["S0"][:])
                d["rc"] = rc
                mu = opool.tile([128, NCH], f32, name="mu", tag="mu")
                nc.vector.tensor_tensor(mu[:], d["S1"][:], rc[:], Alu.mult)
                d["mu"] = mu
                store_half(s, 0, mu)

            def sample_out_sg(s):
                """sg (needs S2) + transpose (PE) + store."""
                d = st[s]
                rc, mu = d["rc"], d["mu"]
                sg = opool.tile([128, NCH], f32, name="sg", tag="sg")
                ex2 = smpool.tile([128, NCH], f32, name="ex2", tag="ex2")
                nc.vector.tensor_tensor(ex2[:], d["S2"][:], rc[:], Alu.mult)
                mu2 = smpool.tile([128, NCH], f32, name="mu2", tag="mu2")
                nc.vector.tensor_tensor(mu2[:], mu[:], mu[:], Alu.mult)
                sg2 = smpool.tile([128, NCH], f32, name="sg2", tag="sg2")
                nc.vector.scalar_tensor_tensor(sg2[:], mu2[:], -1.0, ex2[:],
                                               Alu.mult, Alu.add)
                v2 = smpool.tile([128, NCH], f32, name="v2", tag="v2")
                nc.vector.tensor_scalar(v2[:], sg2[:], 1.0, CLAMP, Alu.mult, Alu.max)
                rsg = smpool.tile([128, NCH], f32, name="rsg", tag="rsg")
                newton_rsqrt(v2[:], rsg[:], NCH, 3)
                nc.vector.tensor_tensor(sg[:], v2[:], rsg[:], Alu.mult)
                store_half(s, 1, sg)

            # ---------------- constant loads (interleaved with x below) ----
            def load_w1xT():
                t = cpool.tile([128, NCH * A], bf16, name="w1xall", tag="w1xall")
                src_ap = w1xT.rearrange("(c p) a -> p c a", p=128)
                nc.sync.dma_start(t[:].rearrange("p (c a) -> p c a", a=A), src_ap)
                return [t[:, c * A:(c + 1) * A] for c in range(NCH)]

            def load_params():
                global b1_t, inv1_t, add1_t, inv2_t, b2p_t, w2T_t, wms_t, ident_t
                b1_t = cpool.tile([A, 1], f32, name="b1", tag="b1")
                nc.sync.dma_start(b1_t[:], b1d[:])
                inv1_t = cpool.tile([A, 1], f32, name="inv1", tag="inv1")
                nc.sync.dma_start(inv1_t[:], inv1d[:])
                add1_t = cpool.tile([A, 1], f32, name="add1", tag="add1")
                nc.sync.dma_start(add1_t[:], add1d[:])
                inv2_t = cpool.tile([128, NCH], f32, name="inv2", tag="inv2")
                nc.sync.dma_start(inv2_t[:], inv2d[:])
                b2p_t = cpool.tile([128, NCH], f32, name="b2p", tag="b2p")
                nc.sync.dma_start(b2p_t[:], b2pd[:])
                w2T_t = cpool.tile([A, C], bf16, name="w2T", tag="w2T")
                nc.sync.dma_start(w2T_t[:], w2T[:])
                ident_t = cpool.tile([128, 128], f32, name="ident", tag="ident")
                nc.sync.dma_start(ident_t[:], identd[:])
                wt = cpool.tile([128, 2 * NCH * A], f32, name="wmsall", tag="wmsall")
                src_ap = wmsT.rearrange("(k p) a -> p k a", p=128)
                nc.sync.dma_start(wt[:].rearrange("p (k a) -> p k a", a=A), src_ap)
                wms_t = [wt[:, k * A:(k + 1) * A] for k in range(2 * NCH)]

            def body():
                global w1xT_t
                # prologue: phase A of samples 0/1, weights interleaved,
                # sample 2's DMA prefetched.  sum(x^2) rotates over
                # DVE/ACT/Pool so no engine serializes the (un-overlapped)
                # prologue.
                dma_x(0, groups=[0], split_first=True)
                w1xT_t = load_w1xT()
                dma_x(0, groups=[1, 2, 3])
                for c in range(NCH):
                    phaseA_moments(0, c, sq=PRO_SQ[c])
                    phaseA_mm1(0, c)
                load_params()
                dma_x(1)
                for c in range(NCH):
                    phaseA_moments(1, c, sq=PRO_SQ[(c + 6) % NCH])
                dma_x(2)
                phaseB(0)
                for c in range(NCH):
                    phaseA_mm1(1, c)
                # steady state: C(s) carries A(s+2) moments, B(s+1) spread
                # over c3/c4/c5/c8, and A(s+2)'s matmul1 in the c>=8 shadow
                # of relu(s+1) freeing the ph1 slot.  ACT's exp runs one
                # chunk ahead of the DVE; S2 accumulation runs one chunk
                # behind (its qt comes from the Pool engine).
                # (s, c) pairs for the exp-lookahead stream, two chunks ahead
                # of the DVE consumer.  g(s) is ready by c==8 of C(s-1), so
                # (s+1, 0) may be emitted from c==10 of C(s) onward.
                mm2exp_seq = [(s, c) for s in range(SPC) for c in range(NCH)]
                mm2exp_pos = 0

                def emit_mm2exp_upto(i):
                    nonlocal mm2exp_pos
                    while mm2exp_pos <= i and mm2exp_pos < len(mm2exp_seq):
                        phaseC_mm2exp(*mm2exp_seq[mm2exp_pos])
                        mm2exp_pos += 1

                emit_mm2exp_upto(1)
                for s in range(SPC):
                    has_a = s + 2 < SPC          # phase-A work interleaved?
                    qt_map = FRONT_QT if has_a else TAIL_QT
                    if s == SPC - 1:
                        qt_map = qt_map[:NCH - 2] + "DD"
                    s1_map = FRONT_S1 if has_a else TAIL_S1
                    for c in range(NCH):
                        emit_mm2exp_upto(s * NCH + c + 2)
                        phaseC_dve(s, c, qt_pool=(qt_map[c] == "P"),
                                   s1_act=(s1_map[c] == "A"))
                        if c > 1:
                            phaseC_s2(s, c - 2)
                        # deferred flush of the previous sample: its last S2s
                        # and the whole output stage run here, where the
                        # Pool's qt and the serial small-op chain can hide
                        # under this sample's chunk stream.
                        if c == 0 and s - 1 in st:
                            phaseC_s2(s - 1, NCH - 2)
                            sample_out_mu(s - 1)
                        if c == 1 and s - 1 in st:
                            phaseC_s2(s - 1, NCH - 1)
                        if c == 2 and s - 1 in st:
                            sample_out_sg(s - 1)
                            del st[s - 1]
                        if s + 3 < SPC and c == 0:
                            dma_x(s + 3)
                        if has_a:
                            if s == 0:
                                # x(2)'s DMA was only issued in the prologue:
                                # shift A(2) two chunks later so the in-order
                                # DVE stream doesn't park on it.
                                if c >= 2:
                                    phaseA_moments(2, c - 2, sq=FRONT_SQ[c - 2])
                            else:
                                phaseA_moments(s + 2, c, sq=FRONT_SQ[c])
                                if s == 1 and c < 2:
                                    phaseA_moments(2, 10 + c, sq=FRONT_SQ[10 + c])
                        bc = (3, 4, 5, 8)
                        if c == bc[0] and s + 1 < SPC:
                            phaseB_stats(s + 1)
                        if c == bc[1] and s + 1 < SPC:
                            phaseB_matvec2(s + 1)
                        if c == bc[2] and s + 1 < SPC:
                            phaseB_relu(s + 1)
                        if c == bc[3] and s + 1 < SPC:
                            phaseB_tanh(s + 1)
                        # mm1(s+2) reuses the single ph1 bank; its first
                        # matmul must be emitted after relu(s+1) has read it.
                        if has_a and c >= 8:
                            for cc in range(3 * (c - 8), 3 * (c - 8) + 3):
                                phaseA_mm1(s + 2, cc)
                s = SPC - 1
                phaseC_s2(s, NCH - 2)
                sample_out_mu(s)
                phaseC_s2(s, NCH - 1)
                sample_out_sg(s)
                del st[s]

            if loop_reps == 1:
                body()
            else:
                with tc.For_i(0, loop_reps, 1):
                    body()

    nc.compile()
    return nc


def _get_module(loop_reps=1):
    key = loop_reps
    if key not in _CACHE:
        _CACHE[key] = _build_module(loop_reps)
    return _CACHE[key]


def _host_prep(inputs):
    """Precompute folded parameters and shard inputs. Returns per-core in_maps."""
    x = np.asarray(inputs["x"])
    W1 = np.asarray(inputs["W1"], np.float32)
    b1 = np.asarray(inputs["b1"], np.float32)
    g1 = np.asarray(inputs["g1"], np.float32)
    beta1 = np.asarray(inputs["beta1"], np.float32)
    rm1 = np.asarray(inputs["rm1"], np.float32)
    rv1 = np.asarray(inputs["rv1"], np.float32)
    W2 = np.asarray(inputs["W2"], np.float32)
    b2 = np.asarray(inputs["b2"], np.float32)
    g2 = np.asarray(inputs["g2"], np.float32)
    rv2 = np.asarray(inputs["rv2"], np.float32)

    inv1 = (g1 / np.sqrt(rv1 + BN_EPS)).astype(np.float32)
    add1 = (beta1 - rm1 * inv1).astype(np.float32)
    inv2 = (g2 / np.sqrt(rv2 + BN_EPS)).astype(np.float32)
    b2p = (inv2 * b2).astype(np.float32)

    const = {
        "w1xT": np.ascontiguousarray(W1[:, :C].T).astype(ml_dtypes.bfloat16),
        "wmsT": np.ascontiguousarray(W1[:, C:].T).astype(np.float32),
        "w2T": np.ascontiguousarray(W2.T).astype(ml_dtypes.bfloat16),
        "b1d": b1.reshape(A, 1),
        "inv1d": inv1.reshape(A, 1),
        "add1d": add1.reshape(A, 1),
        "inv2d": np.ascontiguousarray(inv2.reshape(NCH, 128).T),
        "b2pd": np.ascontiguousarray(b2p.reshape(NCH, 128).T),
        "identd": np.eye(128, dtype=np.float32),
    }
    xbf = x.astype(ml_dtypes.bfloat16)
    in_maps = []
    for core in range(N_CORES):
        m = dict(const)
        m["xbf"] = np.ascontiguousarray(xbf[core * SPC:(core + 1) * SPC])
        in_maps.append(m)
    return in_maps


def kernel(**inputs):
    from concourse.bass_utils import run_bass_kernel_spmd

    nc = _get_module(loop_reps=1)
    in_maps = _host_prep(inputs)
    res = run_bass_kernel_spmd(nc, in_maps, core_ids=list(range(N_CORES)))
    out = np.concatenate([res.results[i]["out"] for i in range(N_CORES)], axis=0)
    return out.astype(np.float32)


# revision 40
# speedup vs baseline: 1245.0508x; 1.0027x over previous
"""AttentiveStatPooling Trainium2 kernel (8-core SPMD, data-parallel over batch).

Contract: kernel(**inputs) takes the FULL unsharded inputs (as produced by
reference.setup_inputs()) and returns the FULL [B, 2C] output.

Math (per sample, identical to the jax reference):
  mean/std over T of x;  h = relu(Wx@x + (Wm@mean + Ws@std + b1));
  g = tanh(BN1(h));  l = BN2scale * relu(W2@g + b2)  (the BN2 shift cancels in
  the softmax and is dropped);  w = softmax(l, axis=T);
  out = [sum(x*w), sqrt(clip(sum(x^2*w) - mu^2, 1e-4))].

Implementation notes (v3 — four-engine balance):
  - batch 32 split 4 samples/core across 8 NeuronCores (pure DP).
  - x shipped in bf16 (halves DMA; all elementwise work runs in bf16 so the
    DVE hits its 2x (tensor_tensor) / 4x (tensor_scalar) perf modes).
  - per-chunk softmax stats with fused accumulates (no standalone reduce):
      eb = max(E,1)  tensor_scalar(max) 4x, fused accum -> S0   [DVE]
      pt = eb*x      tensor_tensor 2x                           [DVE]
      S1 = sum(pt)   tensor_scalar(id) 4x + accum               [DVE or ACT]
      qt = pt*x      tensor_tensor                              [Pool or DVE]
      S2 = sum(qt)   tensor_scalar(id) 4x + accum               [DVE]
    The qt multiply rides the otherwise-idle GPSIMD/Pool engine (plain
    tensor_tensor is the only elementwise op walrus allows there); its S2
    accumulate is deferred two chunks so the DVE never waits on the Pool.
  - x moments per chunk: sum(x) on DVE (tensor_scalar 4x + accum),
    sum(x^2) on ACT (Square+accum) / DVE (x*x + accum) per the *_SQ maps.
    All ACT functions used (Exp/Tanh/Square/Relu/Identity) live in one
    activation table set, so there are no table reloads.
  - softmax needs no max-subtraction (logits bounded, per-row shift cancels);
    relu inside the softmax realized as max(exp(l), 1).
  - sqrt via Newton/rsqrt on the vector engine (avoids ACT table switches),
    3 DVE ops per iteration via scalar_tensor_tensor.
  - emission is software-pipelined: mm2+exp run two chunks ahead of the DVE
    consumer, phase A of sample s+2 and phase B of sample s+1 interleave
    into phase C of sample s, and the previous sample's S2 flush + output
    stage defer into the next sample's chunk stream.  The FRONT_*/TAIL_*/PRO_* knob
    strings balance per-chunk work across DVE/ACT/Pool for the "front"
    samples (which carry phase-A work) and the "tail" samples (which don't).
"""

import numpy as np
import ml_dtypes

B, C, T, A = 32, 1536, 1000, 128
N_CORES = 8
SPC = B // N_CORES        # samples per core
NCH = C // 128            # 12 channel chunks of 128
BN_EPS = 1e-5
CLAMP = 1e-4
HALVES = ((0, 512), (512, 1000))   # psum-bank-aligned split of T

_CACHE = {}

# Engine-balance knobs (index = c % 12).  For each chunk of "front" samples
# (those that also carry phase-A work of sample s+2) and "tail" samples:
#   qt engine: 'P' (Pool) or 'D' (DVE)
#   S1 accumulate: 'D' (DVE tensor_scalar) or 'A' (ACT identity+accum)
#   phase-A sum(x^2): 'A' (ACT Square), 'D' (DVE x*x+accum), 'P' (Pool x*x
#   + DVE accum)
FRONT_QT = "PPPPPPPPPPPP"
FRONT_S1 = "DDDDDDDDDDDD"
FRONT_SQ = "AAADAAADAAAD"
TAIL_QT = "DPPPDPPPDPPP"
TAIL_S1 = "ADADDADADDAD"
PRO_SQ = "DADADADADADA"
FRONT_MX = "DDDDDDDDDDDD"   # prologue samples 0/1 (rotated by 6 for s=1)


def _build_module(loop_reps=1):
    import concourse.tile as tile
    from concourse import bacc, mybir
    from contextlib import ExitStack

    f32, bf16 = mybir.dt.float32, mybir.dt.bfloat16
    Alu = mybir.AluOpType
    Act = mybir.ActivationFunctionType

    nc = bacc.Bacc("TRN2", target_bir_lowering=False, debug=False,
                   num_devices=N_CORES)

    xbf = nc.dram_tensor("xbf", [SPC, C, T], bf16, kind="ExternalInput").ap()
    w1xT = nc.dram_tensor("w1xT", [C, A], bf16, kind="ExternalInput").ap()
    wmsT = nc.dram_tensor("wmsT", [2 * C, A], f32, kind="ExternalInput").ap()
    w2T = nc.dram_tensor("w2T", [A, C], bf16, kind="ExternalInput").ap()
    b1d = nc.dram_tensor("b1d", [A, 1], f32, kind="ExternalInput").ap()
    inv1d = nc.dram_tensor("inv1d", [A, 1], f32, kind="ExternalInput").ap()
    add1d = nc.dram_tensor("add1d", [A, 1], f32, kind="ExternalInput").ap()
    inv2d = nc.dram_tensor("inv2d", [128, NCH], f32, kind="ExternalInput").ap()
    b2pd = nc.dram_tensor("b2pd", [128, NCH], f32, kind="ExternalInput").ap()
    identd = nc.dram_tensor("identd", [128, 128], f32, kind="ExternalInput").ap()
    out = nc.dram_tensor("out", [SPC, 2 * C], f32, kind="ExternalOutput").ap()

    with tile.TileContext(nc) as tc:
        with ExitStack() as ctx:
            cpool = ctx.enter_context(tc.tile_pool(name="const", bufs=1))
            xpool = ctx.enter_context(tc.tile_pool(name="x", bufs=14))
            epool = ctx.enter_context(tc.tile_pool(name="e", bufs=3))
            ebpool = ctx.enter_context(tc.tile_pool(name="eb", bufs=3))
            ppool = ctx.enter_context(tc.tile_pool(name="p", bufs=3))
            qpool = ctx.enter_context(tc.tile_pool(name="q", bufs=4))
            jpool = ctx.enter_context(tc.tile_pool(name="junk", bufs=6))
            rpool = ctx.enter_context(tc.tile_pool(name="r", bufs=2))
            gpool = ctx.enter_context(tc.tile_pool(name="g", bufs=2))
            spool = ctx.enter_context(tc.tile_pool(name="stats", bufs=4))
            smpool = ctx.enter_context(tc.tile_pool(name="small", bufs=8))
            opool = ctx.enter_context(tc.tile_pool(name="ostage", bufs=4))
            ph1p = ctx.enter_context(tc.tile_pool(name="ph1", bufs=1, space="PSUM"))
            p2p = ctx.enter_context(tc.tile_pool(name="p2", bufs=2, space="PSUM"))
            pmvp = ctx.enter_context(tc.tile_pool(name="pmv", bufs=1, space="PSUM"))
            ptrp = ctx.enter_context(tc.tile_pool(name="ptr", bufs=1, space="PSUM"))

            st = {}   # per-sample state

            def dma_x(s, groups=range(4), split_first=False):
                if s not in st:
                    st[s] = {"xg": [], "x": []}
                for g in groups:
                    xt = xpool.tile([128, 3 * T], bf16, name="x", tag="x")
                    src_ap = xbf[s, g * 384:(g + 1) * 384, :]
                    src_ap = src_ap.rearrange("(c p) t -> p c t", p=128)
                    dst = xt[:].rearrange("p (c t) -> p c t", t=T)
                    if split_first and g == 0:
                        # land chunk 0 first so phase A can start sooner
                        nc.sync.dma_start(dst[:, 0:1], src_ap[:, 0:1])
                        nc.sync.dma_start(dst[:, 1:3], src_ap[:, 1:3])
                    else:
                        nc.sync.dma_start(dst, src_ap)
                    st[s]["xg"].append(xt)
                    for i in range(3):
                        st[s]["x"].append(xt[:, i * T:(i + 1) * T])

            def phaseA_moments(s, c, sq="A"):
                """sum(x) on DVE (4x) + sum(x^2) on ACT (Square+accum), DVE
                (x*x then accum), or Pool (x*x) + DVE accum."""
                d = st[s]
                if c == 0:
                    d["Mx"] = spool.tile([128, NCH], f32, name="Mx", tag="Mx")
                    d["Mx2"] = spool.tile([128, NCH], f32, name="Mx2", tag="Mx2")
                xt = d["x"][c]
                j0 = jpool.tile([128, T], bf16, name="junk", tag="junk")
                if FRONT_MX[c] == "A":
                    nc.scalar.activation(j0[:], xt, Act.Identity,
                                         accum_out=d["Mx"][:, c:c + 1])
                else:
                    nc.vector.tensor_scalar(j0[:], xt, 0.0, 0.0, Alu.add,
                                            Alu.add,
                                            accum_out=d["Mx"][:, c:c + 1])
                if sq == "A":
                    j1 = jpool.tile([128, T], bf16, name="junk", tag="junk")
                    nc.scalar.activation(j1[:], xt, Act.Square,
                                         accum_out=d["Mx2"][:, c:c + 1])
                else:
                    x2 = jpool.tile([128, T], bf16, name="junk", tag="junk")
                    if sq == "P":
                        nc.gpsimd.tensor_tensor(x2[:], xt, xt, Alu.mult)
                    else:
                        nc.vector.tensor_tensor(x2[:], xt, xt, Alu.mult)
                    j1 = jpool.tile([128, T], bf16, name="junk", tag="junk")
                    nc.vector.tensor_scalar(j1[:], x2[:], 0.0, 0.0, Alu.add,
                                            Alu.add,
                                            accum_out=d["Mx2"][:, c:c + 1])

            def phaseA_mm1(s, c):
                d = st[s]
                if c == 0:
                    d["ph1"] = ph1p.tile([A, T], f32, name="ph1", tag="ph1")
                xt = d["x"][c]
                for lo, hi in HALVES:
                    nc.tensor.matmul(d["ph1"][:, lo:hi], w1xT_t[c],
                                     xt[:, lo:hi], start=(c == 0),
                                     stop=(c == NCH - 1), skip_group_check=True)

            def newton_rsqrt(v_ap, out_ap, n, iters):
                """out = 1/sqrt(v) elementwise on a [128, n] fp32 AP.
                Seed r0 = 2/(1+v) (robust for any v>0), then Newton
                iterations r' = r*(1.5 - 0.5*v*r^2), 3 DVE ops each:
                  t = r*r;  u = (t*-0.5)*v;  r' = (u+1.5)*r."""
                t0 = smpool.tile([128, n], f32, name="nw0", tag="nw0")
                t1 = smpool.tile([128, n], f32, name="nw1", tag="nw1")
                r = smpool.tile([128, n], f32, name="nwr", tag="nwr")
                nc.vector.tensor_scalar(t0[:], v_ap, 0.5, 0.5, Alu.mult, Alu.add)
                nc.vector.reciprocal(r[:], t0[:])
                for it in range(iters):
                    dst = out_ap if it == iters - 1 else r[:]
                    nc.vector.tensor_tensor(t0[:], r[:], r[:], Alu.mult)
                    nc.vector.scalar_tensor_tensor(t1[:], t0[:], -0.5, v_ap,
                                                   Alu.mult, Alu.mult)
                    nc.vector.scalar_tensor_tensor(dst, t1[:], 1.5, r[:],
                                                   Alu.add, Alu.mult)

            def phaseB_stats(s):
                """mean/std from Mx/Mx2 + mean-half of the bias matvec."""
                d = st[s]
                meanc = smpool.tile([128, NCH], f32, name="meanc", tag="meanc")
                nc.vector.tensor_scalar(meanc[:], d["Mx"][:], 1.0 / T, None, Alu.mult)
                pmv = pmvp.tile([A, 1], f32, name="pmv", tag="pmv")
                d["pmv"] = pmv
                for k in range(NCH):
                    nc.tensor.matmul(pmv[:], wms_t[k], meanc[:, k:k + 1],
                                     start=(k == 0), stop=False,
                                     skip_group_check=True)
                # unbiased var = (Mx2 - T*mean^2) / (T-1);  T*mean^2 = mean*Mx
                tm2 = smpool.tile([128, NCH], f32, name="tm2", tag="tm2")
                nc.vector.tensor_tensor(tm2[:], meanc[:], d["Mx"][:], Alu.mult)
                vdiff = smpool.tile([128, NCH], f32, name="vdiff", tag="vdiff")
                nc.vector.scalar_tensor_tensor(vdiff[:], tm2[:], -1.0, d["Mx2"][:],
                                               Alu.mult, Alu.add)
                v = smpool.tile([128, NCH], f32, name="v", tag="v")
                nc.vector.tensor_scalar(v[:], vdiff[:], 1.0 / (T - 1.0), CLAMP,
                                        Alu.mult, Alu.max)
                std_t = smpool.tile([128, NCH], f32, name="std_t", tag="std_t")
                d["std_t"] = std_t
                rs = smpool.tile([128, NCH], f32, name="rs", tag="rs")
                newton_rsqrt(v[:], rs[:], NCH, 2)
                nc.vector.tensor_tensor(std_t[:], v[:], rs[:], Alu.mult)

            def phaseB_matvec2(s):
                """std-half of the matvec + btot."""
                d = st[s]
                pmv, std_t = d["pmv"], d["std_t"]
                for k in range(NCH):
                    nc.tensor.matmul(pmv[:], wms_t[NCH + k], std_t[:, k:k + 1],
                                     start=False, stop=(k == NCH - 1),
                                     skip_group_check=True)
                btot = smpool.tile([A, 1], f32, name="btot", tag="btot")
                nc.vector.tensor_tensor(btot[:], pmv[:], b1_t[:], Alu.add)
                d["btot"] = btot

            def phaseB_relu(s):
                d = st[s]
                rt = rpool.tile([A, T], bf16, name="r", tag="r")
                nc.scalar.activation(rt[:], d["ph1"][:], Act.Relu, bias=d["btot"][:])
                d["rt"] = rt

            def phaseB_tanh(s):
                d = st[s]
                gt = gpool.tile([A, T], bf16, name="g", tag="g")
                nc.scalar.activation(gt[:], d["rt"][:], Act.Tanh, bias=add1_t[:],
                                     scale=inv1_t[:])
                d["g"] = gt

            def phaseB(s):
                phaseB_stats(s)
                phaseB_matvec2(s)
                phaseB_relu(s)
                phaseB_tanh(s)

            def phaseC_mm2exp(s, c):
                """PE matmul2 + ACT exp for chunk c (emitted one chunk ahead
                of the DVE consumer so the in-order ACT stream never starves
                the DVE)."""
                d = st[s]
                if c == 0:
                    d["S0"] = spool.tile([128, NCH], f32, name="S0", tag="S0")
                    d["S1"] = spool.tile([128, NCH], f32, name="S1", tag="S1")
                    d["S2"] = spool.tile([128, NCH], f32, name="S2", tag="S2")
                    d["E"] = [None] * NCH
                p2 = p2p.tile([128, T], f32, name="p2", tag="p2")
                wsl = w2T_t[:, c * 128:(c + 1) * 128]
                for lo, hi in HALVES:
                    nc.tensor.matmul(p2[:, lo:hi], wsl, d["g"][:, lo:hi],
                                     start=True, stop=True)
                E = epool.tile([128, T], bf16, name="E", tag="E")
                nc.scalar.activation(E[:], p2[:], Act.Exp,
                                     bias=b2p_t[:, c:c + 1], scale=inv2_t[:, c:c + 1])
                d["E"][c] = E

            def phaseC_dve(s, c, qt_pool=True, s1_act=False):
                """eb/pt/S1 for chunk c; qt on Pool (or DVE); S2 of chunk c-2
                (deferred two chunks so the DVE never waits on Pool's qt)."""
                d = st[s]
                E = d["E"][c]
                d["E"][c] = None
                eb = ebpool.tile([128, T], bf16, name="eb", tag="eb")
                nc.vector.tensor_scalar(eb[:], E[:], 1.0, 0.0, Alu.max, Alu.add,
                                        accum_out=d["S0"][:, c:c + 1])
                xt = d["x"][c]
                pt = ppool.tile([128, T], bf16, name="p", tag="p")
                nc.vector.tensor_tensor(pt[:], eb[:], xt, Alu.mult)
                qt = qpool.tile([128, T], bf16, name="q", tag="q")
                if qt_pool:
                    nc.gpsimd.tensor_tensor(qt[:], pt[:], xt, Alu.mult)
                else:
                    nc.vector.tensor_tensor(qt[:], pt[:], xt, Alu.mult)
                d["qt_%d" % c] = qt
                if s1_act:
                    j1 = jpool.tile([128, T], bf16, name="junk", tag="junk")
                    nc.scalar.activation(j1[:], pt[:], Act.Identity,
                                         accum_out=d["S1"][:, c:c + 1])
                else:
                    j1 = jpool.tile([128, T], bf16, name="junk", tag="junk")
                    nc.vector.tensor_scalar(j1[:], pt[:], 0.0, 0.0, Alu.add,
                                            Alu.add,
                                            accum_out=d["S1"][:, c:c + 1])

            def phaseC_s2(s, c):
                """S2 accumulate for chunk c (reads qt produced on Pool)."""
                d = st[s]
                qt = d.pop("qt_%d" % c)
                j2 = jpool.tile([128, T], bf16, name="junk", tag="junk")
                nc.vector.tensor_scalar(j2[:], qt[:], 0.0, 0.0, Alu.add, Alu.add,
                                        accum_out=d["S2"][:, c:c + 1])

            def store_half(s, half, srct):
                ptr = ptrp.tile([NCH, 128], f32, name="ptr", tag="ptr")
                nc.tensor.transpose(ptr[:], srct[:], ident_t[:])
                ost = opool.tile([NCH, 128], f32, name="ost", tag="ost")
                nc.scalar.copy(ost[:], ptr[:])
                dst = out[s, half * C:(half + 1) * C]
                dst = dst.rearrange("(ci p) -> ci p", p=128)
                nc.sync.dma_start(dst, ost[:])

            def sample_out_mu(s):
                """mu (needs only S0/S1) + transpose (PE) + store."""
                d = st[s]
                rc = smpool.tile([128, NCH], f32, name="rc", tag="rc")
                nc.vector.reciprocal(rc[:], d["S0"][:])
                d["rc"] = rc
                mu = opool.tile([128, NCH], f32, name="mu", tag="mu")
                nc.vector.tensor_tensor(mu[:], d["S1"][:], rc[:], Alu.mult)
                d["mu"] = mu
                store_half(s, 0, mu)

            def sample_out_sg(s):
                """sg (needs S2) + transpose (PE) + store."""
                d = st[s]
                rc, mu = d["rc"], d["mu"]
                sg = opool.tile([128, NCH], f32, name="sg", tag="sg")
                ex2 = smpool.tile([128, NCH], f32, name="ex2", tag="ex2")
                nc.vector.tensor_tensor(ex2[:], d["S2"][:], rc[:], Alu.mult)
                mu2 = smpool.tile([128, NCH], f32, name="mu2", tag="mu2")
                nc.vector.tensor_tensor(mu2[:], mu[:], mu[:], Alu.mult)
                sg2 = smpool.tile([128, NCH], f32, name="sg2", tag="sg2")
                nc.vector.scalar_tensor_tensor(sg2[:], mu2[:], -1.0, ex2[:],
                                               Alu.mult, Alu.add)
                v2 = smpool.tile([128, NCH], f32, name="v2", tag="v2")
                nc.vector.tensor_scalar(v2[:], sg2[:], 1.0, CLAMP, Alu.mult, Alu.max)
                rsg = smpool.tile([128, NCH], f32, name="rsg", tag="rsg")
                newton_rsqrt(v2[:], rsg[:], NCH, 3)
                nc.vector.tensor_tensor(sg[:], v2[:], rsg[:], Alu.mult)
                store_half(s, 1, sg)

            # ---------------- constant loads (interleaved with x below) ----
            def load_w1xT():
                t = cpool.tile([128, NCH * A], bf16, name="w1xall", tag="w1xall")
                src_ap = w1xT.rearrange("(c p) a -> p c a", p=128)
                nc.sync.dma_start(t[:].rearrange("p (c a) -> p c a", a=A), src_ap)
                return [t[:, c * A:(c + 1) * A] for c in range(NCH)]

            def load_params():
                global b1_t, inv1_t, add1_t, inv2_t, b2p_t, w2T_t, wms_t, ident_t
                b1_t = cpool.tile([A, 1], f32, name="b1", tag="b1")
                nc.sync.dma_start(b1_t[:], b1d[:])
                inv1_t = cpool.tile([A, 1], f32, name="inv1", tag="inv1")
                nc.sync.dma_start(inv1_t[:], inv1d[:])
                add1_t = cpool.tile([A, 1], f32, name="add1", tag="add1")
                nc.sync.dma_start(add1_t[:], add1d[:])
                inv2_t = cpool.tile([128, NCH], f32, name="inv2", tag="inv2")
                nc.sync.dma_start(inv2_t[:], inv2d[:])
                b2p_t = cpool.tile([128, NCH], f32, name="b2p", tag="b2p")
                nc.sync.dma_start(b2p_t[:], b2pd[:])
                w2T_t = cpool.tile([A, C], bf16, name="w2T", tag="w2T")
                nc.sync.dma_start(w2T_t[:], w2T[:])
                ident_t = cpool.tile([128, 128], f32, name="ident", tag="ident")
                nc.sync.dma_start(ident_t[:], identd[:])
                wt = cpool.tile([128, 2 * NCH * A], f32, name="wmsall", tag="wmsall")
                src_ap = wmsT.rearrange("(k p) a -> p k a", p=128)
                nc.sync.dma_start(wt[:].rearrange("p (k a) -> p k a", a=A), src_ap)
                wms_t = [wt[:, k * A:(k + 1) * A] for k in range(2 * NCH)]

            def body():
                global w1xT_t
                # prologue: phase A of samples 0/1, weights interleaved,
                # sample 2's DMA prefetched.  sum(x^2) rotates over
                # DVE/ACT/Pool so no engine serializes the (un-overlapped)
                # prologue.
                dma_x(0, groups=[0], split_first=True)
                w1xT_t = load_w1xT()
                dma_x(0, groups=[1, 2, 3])
                for c in range(NCH):
                    phaseA_moments(0, c, sq=PRO_SQ[c])
                    phaseA_mm1(0, c)
                load_params()
                dma_x(1)
                for c in range(NCH):
                    phaseA_moments(1, c, sq=PRO_SQ[(c + 6) % NCH])
                dma_x(2)
                phaseB(0)
                for c in range(NCH):
                    phaseA_mm1(1, c)
                # steady state: C(s) carries A(s+2) moments, B(s+1) spread
                # over c3/c4/c5/c8, and A(s+2)'s matmul1 in the c>=8 shadow
                # of relu(s+1) freeing the ph1 slot.  ACT's exp runs one
                # chunk ahead of the DVE; S2 accumulation runs one chunk
                # behind (its qt comes from the Pool engine).
                # (s, c) pairs for the exp-lookahead stream, two chunks ahead
                # of the DVE consumer.  g(s) is ready by c==8 of C(s-1), so
                # (s+1, 0) may be emitted from c==10 of C(s) onward.
                mm2exp_seq = [(s, c) for s in range(SPC) for c in range(NCH)]
                mm2exp_pos = 0

                def emit_mm2exp_upto(i):
                    nonlocal mm2exp_pos
                    while mm2exp_pos <= i and mm2exp_pos < len(mm2exp_seq):
                        phaseC_mm2exp(*mm2exp_seq[mm2exp_pos])
                        mm2exp_pos += 1

                emit_mm2exp_upto(1)
                for s in range(SPC):
                    has_a = s + 2 < SPC          # phase-A work interleaved?
                    qt_map = FRONT_QT if has_a else TAIL_QT
                    if s == SPC - 1:
                        qt_map = qt_map[:NCH - 2] + "DD"
                    s1_map = FRONT_S1 if has_a else TAIL_S1
                    for c in range(NCH):
                        emit_mm2exp_upto(s * NCH + c + 2)
                        phaseC_dve(s, c, qt_pool=(qt_map[c] == "P"),
                                   s1_act=(s1_map[c] == "A"))
                        if c > 1:
                            phaseC_s2(s, c - 2)
                        # deferred flush of the previous sample: its last S2s
                        # and the whole output stage run here, where the
                        # Pool's qt and the serial small-op chain can hide
                        # under this sample's chunk stream.
                        if c == 0 and s - 1 in st:
                            phaseC_s2(s - 1, NCH - 2)
                            sample_out_mu(s - 1)
                        if c == 1 and s - 1 in st:
                            phaseC_s2(s - 1, NCH - 1)
                        if c == 2 and s - 1 in st:
                            sample_out_sg(s - 1)
                            del st[s - 1]
                        if s + 3 < SPC and c == 0:
                            dma_x(s + 3)
                        if s == 2 and c < 4:
                            phaseA_moments(3, 8 + c, sq=FRONT_SQ[8 + c])
                        if has_a:
                            if s == 0:
                                # x(2)'s DMA was only issued in the prologue:
                                # shift A(2) two chunks later so the in-order
                                # DVE stream doesn't park on it.
                                if c >= 2:
                                    phaseA_moments(2, c - 2, sq=FRONT_SQ[c - 2])
                            elif s == 1:
                                # A(3) spills 4 chunks into C(2) to even the
                                # front/tail engine loads.
                                if c < 8:
                                    phaseA_moments(3, c, sq=FRONT_SQ[c])
                                if c < 2:
                                    phaseA_moments(2, 10 + c, sq=FRONT_SQ[10 + c])
                            else:
                                phaseA_moments(s + 2, c, sq=FRONT_SQ[c])
                        bc = (5, 6, 7, 8) if s == 2 else (3, 4, 5, 8)
                        if c == bc[0] and s + 1 < SPC:
                            phaseB_stats(s + 1)
                        if c == bc[1] and s + 1 < SPC:
                            phaseB_matvec2(s + 1)
                        if c == bc[2] and s + 1 < SPC:
                            phaseB_relu(s + 1)
                        if c == bc[3] and s + 1 < SPC:
                            phaseB_tanh(s + 1)
                        # mm1(s+2) reuses the single ph1 bank; its first
                        # matmul must be emitted after relu(s+1) has read it.
                        if has_a and c >= 8:
                            for cc in range(3 * (c - 8), 3 * (c - 8) + 3):
                                phaseA_mm1(s + 2, cc)
                s = SPC - 1
                phaseC_s2(s, NCH - 2)
                sample_out_mu(s)
                phaseC_s2(s, NCH - 1)
                sample_out_sg(s)
                del st[s]

            if loop_reps == 1:
                body()
            else:
                with tc.For_i(0, loop_reps, 1):
                    body()

    nc.compile()
    return nc


def _get_module(loop_reps=1):
    key = loop_reps
    if key not in _CACHE:
        _CACHE[key] = _build_module(loop_reps)
    return _CACHE[key]


def _host_prep(inputs):
    """Precompute folded parameters and shard inputs. Returns per-core in_maps."""
    x = np.asarray(inputs["x"])
    W1 = np.asarray(inputs["W1"], np.float32)
    b1 = np.asarray(inputs["b1"], np.float32)
    g1 = np.asarray(inputs["g1"], np.float32)
    beta1 = np.asarray(inputs["beta1"], np.float32)
    rm1 = np.asarray(inputs["rm1"], np.float32)
    rv1 = np.asarray(inputs["rv1"], np.float32)
    W2 = np.asarray(inputs["W2"], np.float32)
    b2 = np.asarray(inputs["b2"], np.float32)
    g2 = np.asarray(inputs["g2"], np.float32)
    rv2 = np.asarray(inputs["rv2"], np.float32)

    inv1 = (g1 / np.sqrt(rv1 + BN_EPS)).astype(np.float32)
    add1 = (beta1 - rm1 * inv1).astype(np.float32)
    inv2 = (g2 / np.sqrt(rv2 + BN_EPS)).astype(np.float32)
    b2p = (inv2 * b2).astype(np.float32)

    const = {
        "w1xT": np.ascontiguousarray(W1[:, :C].T).astype(ml_dtypes.bfloat16),
        "wmsT": np.ascontiguousarray(W1[:, C:].T).astype(np.float32),
        "w2T": np.ascontiguousarray(W2.T).astype(ml_dtypes.bfloat16),
        "b1d": b1.reshape(A, 1),
        "inv1d": inv1.reshape(A, 1),
        "add1d": add1.reshape(A, 1),
        "inv2d": np.ascontiguousarray(inv2.reshape(NCH, 128).T),
        "b2pd": np.ascontiguousarray(b2p.reshape(NCH, 128).T),
        "identd": np.eye(128, dtype=np.float32),
    }
    xbf = x.astype(ml_dtypes.bfloat16)
    in_maps = []
    for core in range(N_CORES):
        m = dict(const)
        m["xbf"] = np.ascontiguousarray(xbf[core * SPC:(core + 1) * SPC])
        in_maps.append(m)
    return in_maps


def kernel(**inputs):
    from concourse.bass_utils import run_bass_kernel_spmd

    nc = _get_module(loop_reps=1)
    in_maps = _host_prep(inputs)
    res = run_bass_kernel_spmd(nc, in_maps, core_ids=list(range(N_CORES)))
    out = np.concatenate([res.results[i]["out"] for i in range(N_CORES)], axis=0)
    return out.astype(np.float32)


# revision 41
# speedup vs baseline: 1253.1638x; 1.0065x over previous
"""AttentiveStatPooling Trainium2 kernel (8-core SPMD, data-parallel over batch).

Contract: kernel(**inputs) takes the FULL unsharded inputs (as produced by
reference.setup_inputs()) and returns the FULL [B, 2C] output.

Math (per sample, identical to the jax reference):
  mean/std over T of x;  h = relu(Wx@x + (Wm@mean + Ws@std + b1));
  g = tanh(BN1(h));  l = BN2scale * relu(W2@g + b2)  (the BN2 shift cancels in
  the softmax and is dropped);  w = softmax(l, axis=T);
  out = [sum(x*w), sqrt(clip(sum(x^2*w) - mu^2, 1e-4))].

Implementation notes (v3 — four-engine balance):
  - batch 32 split 4 samples/core across 8 NeuronCores (pure DP).
  - x shipped in bf16 (halves DMA; all elementwise work runs in bf16 so the
    DVE hits its 2x (tensor_tensor) / 4x (tensor_scalar) perf modes).
  - per-chunk softmax stats with fused accumulates (no standalone reduce):
      eb = max(E,1)  tensor_scalar(max) 4x, fused accum -> S0   [DVE]
      pt = eb*x      tensor_tensor 2x                           [DVE]
      S1 = sum(pt)   tensor_scalar(id) 4x + accum               [DVE or ACT]
      qt = pt*x      tensor_tensor                              [Pool or DVE]
      S2 = sum(qt)   tensor_scalar(id) 4x + accum               [DVE]
    The qt multiply rides the otherwise-idle GPSIMD/Pool engine (plain
    tensor_tensor is the only elementwise op walrus allows there); its S2
    accumulate is deferred two chunks so the DVE never waits on the Pool.
  - x moments per chunk: sum(x) on DVE (tensor_scalar 4x + accum),
    sum(x^2) on ACT (Square+accum) / DVE (x*x + accum) per the *_SQ maps.
    All ACT functions used (Exp/Tanh/Square/Relu/Identity) live in one
    activation table set, so there are no table reloads.
  - softmax needs no max-subtraction (logits bounded, per-row shift cancels);
    relu inside the softmax realized as max(exp(l), 1).
  - sqrt via Newton/rsqrt on the vector engine (avoids ACT table switches),
    3 DVE ops per iteration via scalar_tensor_tensor.
  - emission is software-pipelined: mm2+exp run two chunks ahead of the DVE
    consumer, phase A of sample s+2 and phase B of sample s+1 interleave
    into phase C of sample s, and the previous sample's S2 flush + output
    stage defer into the next sample's chunk stream.  The FRONT_*/TAIL_*/PRO_* knob
    strings balance per-chunk work across DVE/ACT/Pool for the "front"
    samples (which carry phase-A work) and the "tail" samples (which don't).
"""

import numpy as np
import ml_dtypes

B, C, T, A = 32, 1536, 1000, 128
N_CORES = 8
SPC = B // N_CORES        # samples per core
NCH = C // 128            # 12 channel chunks of 128
BN_EPS = 1e-5
CLAMP = 1e-4
HALVES = ((0, 512), (512, 1000))   # psum-bank-aligned split of T

_CACHE = {}

# Engine-balance knobs (index = c % 12).  For each chunk of "front" samples
# (those that also carry phase-A work of sample s+2) and "tail" samples:
#   qt engine: 'P' (Pool) or 'D' (DVE)
#   S1 accumulate: 'D' (DVE tensor_scalar) or 'A' (ACT identity+accum)
#   phase-A sum(x^2): 'A' (ACT Square), 'D' (DVE x*x+accum), 'P' (Pool x*x
#   + DVE accum)
FRONT_QT = "PPPPPPPPPPPP"
FRONT_S1 = "DDDDDDDDDDDD"
FRONT_SQ = "AAADAAADAAAD"
TAIL_QT = "DPPPDPPPDPPP"
TAIL_S1 = "ADADDADADDAD"
PRO_SQ = "DADADADADADA"
FRONT_MX = "DDDDDDDDDDDD"   # prologue samples 0/1 (rotated by 6 for s=1)


def _build_module(loop_reps=1):
    import concourse.tile as tile
    from concourse import bacc, mybir
    from contextlib import ExitStack

    f32, bf16 = mybir.dt.float32, mybir.dt.bfloat16
    Alu = mybir.AluOpType
    Act = mybir.ActivationFunctionType

    nc = bacc.Bacc("TRN2", target_bir_lowering=False, debug=False,
                   num_devices=N_CORES)

    xbf = nc.dram_tensor("xbf", [SPC, C, T], bf16, kind="ExternalInput").ap()
    w1xT = nc.dram_tensor("w1xT", [C, A], bf16, kind="ExternalInput").ap()
    wmsT = nc.dram_tensor("wmsT", [2 * C, A], f32, kind="ExternalInput").ap()
    w2T = nc.dram_tensor("w2T", [A, C], bf16, kind="ExternalInput").ap()
    b1d = nc.dram_tensor("b1d", [A, 1], f32, kind="ExternalInput").ap()
    inv1d = nc.dram_tensor("inv1d", [A, 1], f32, kind="ExternalInput").ap()
    add1d = nc.dram_tensor("add1d", [A, 1], f32, kind="ExternalInput").ap()
    inv2d = nc.dram_tensor("inv2d", [128, NCH], f32, kind="ExternalInput").ap()
    b2pd = nc.dram_tensor("b2pd", [128, NCH], f32, kind="ExternalInput").ap()
    identd = nc.dram_tensor("identd", [128, 128], f32, kind="ExternalInput").ap()
    out = nc.dram_tensor("out", [SPC, 2 * C], f32, kind="ExternalOutput").ap()

    with tile.TileContext(nc) as tc:
        with ExitStack() as ctx:
            cpool = ctx.enter_context(tc.tile_pool(name="const", bufs=1))
            xpool = ctx.enter_context(tc.tile_pool(name="x", bufs=14))
            epool = ctx.enter_context(tc.tile_pool(name="e", bufs=3))
            ebpool = ctx.enter_context(tc.tile_pool(name="eb", bufs=3))
            ppool = ctx.enter_context(tc.tile_pool(name="p", bufs=3))
            qpool = ctx.enter_context(tc.tile_pool(name="q", bufs=4))
            jpool = ctx.enter_context(tc.tile_pool(name="junk", bufs=6))
            rpool = ctx.enter_context(tc.tile_pool(name="r", bufs=2))
            gpool = ctx.enter_context(tc.tile_pool(name="g", bufs=2))
            spool = ctx.enter_context(tc.tile_pool(name="stats", bufs=4))
            smpool = ctx.enter_context(tc.tile_pool(name="small", bufs=8))
            opool = ctx.enter_context(tc.tile_pool(name="ostage", bufs=4))
            ph1p = ctx.enter_context(tc.tile_pool(name="ph1", bufs=1, space="PSUM"))
            p2p = ctx.enter_context(tc.tile_pool(name="p2", bufs=2, space="PSUM"))
            pmvp = ctx.enter_context(tc.tile_pool(name="pmv", bufs=1, space="PSUM"))
            ptrp = ctx.enter_context(tc.tile_pool(name="ptr", bufs=1, space="PSUM"))

            st = {}   # per-sample state

            def dma_x(s, groups=range(4), split_first=False):
                if s not in st:
                    st[s] = {"xg": [], "x": []}
                for g in groups:
                    xt = xpool.tile([128, 3 * T], bf16, name="x", tag="x")
                    src_ap = xbf[s, g * 384:(g + 1) * 384, :]
                    src_ap = src_ap.rearrange("(c p) t -> p c t", p=128)
                    dst = xt[:].rearrange("p (c t) -> p c t", t=T)
                    if split_first and g == 0:
                        # land chunk 0 first so phase A can start sooner
                        nc.sync.dma_start(dst[:, 0:1], src_ap[:, 0:1])
                        nc.sync.dma_start(dst[:, 1:3], src_ap[:, 1:3])
                    else:
                        nc.sync.dma_start(dst, src_ap)
                    st[s]["xg"].append(xt)
                    for i in range(3):
                        st[s]["x"].append(xt[:, i * T:(i + 1) * T])

            def phaseA_moments(s, c, sq="A"):
                """sum(x) on DVE (4x) + sum(x^2) on ACT (Square+accum), DVE
                (x*x then accum), or Pool (x*x) + DVE accum."""
                d = st[s]
                if c == 0:
                    d["Mx"] = spool.tile([128, NCH], f32, name="Mx", tag="Mx")
                    d["Mx2"] = spool.tile([128, NCH], f32, name="Mx2", tag="Mx2")
                xt = d["x"][c]
                j0 = jpool.tile([128, T], bf16, name="junk", tag="junk")
                if FRONT_MX[c] == "A":
                    nc.scalar.activation(j0[:], xt, Act.Identity,
                                         accum_out=d["Mx"][:, c:c + 1])
                else:
                    nc.vector.tensor_scalar(j0[:], xt, 0.0, 0.0, Alu.add,
                                            Alu.add,
                                            accum_out=d["Mx"][:, c:c + 1])
                if sq == "A":
                    j1 = jpool.tile([128, T], bf16, name="junk", tag="junk")
                    nc.scalar.activation(j1[:], xt, Act.Square,
                                         accum_out=d["Mx2"][:, c:c + 1])
                else:
                    x2 = jpool.tile([128, T], bf16, name="junk", tag="junk")
                    if sq == "P":
                        nc.gpsimd.tensor_tensor(x2[:], xt, xt, Alu.mult)
                    else:
                        nc.vector.tensor_tensor(x2[:], xt, xt, Alu.mult)
                    j1 = jpool.tile([128, T], bf16, name="junk", tag="junk")
                    nc.vector.tensor_scalar(j1[:], x2[:], 0.0, 0.0, Alu.add,
                                            Alu.add,
                                            accum_out=d["Mx2"][:, c:c + 1])

            def phaseA_mm1(s, c):
                d = st[s]
                if c == 0:
                    d["ph1"] = ph1p.tile([A, T], f32, name="ph1", tag="ph1")
                xt = d["x"][c]
                for lo, hi in HALVES:
                    nc.tensor.matmul(d["ph1"][:, lo:hi], w1xT_t[c],
                                     xt[:, lo:hi], start=(c == 0),
                                     stop=(c == NCH - 1), skip_group_check=True)

            def newton_rsqrt(v_ap, out_ap, n, iters):
                """out = 1/sqrt(v) elementwise on a [128, n] fp32 AP.
                Seed r0 = 2/(1+v) (robust for any v>0), then Newton
                iterations r' = r*(1.5 - 0.5*v*r^2), 3 DVE ops each:
                  t = r*r;  u = (t*-0.5)*v;  r' = (u+1.5)*r."""
                t0 = smpool.tile([128, n], f32, name="nw0", tag="nw0")
                t1 = smpool.tile([128, n], f32, name="nw1", tag="nw1")
                r = smpool.tile([128, n], f32, name="nwr", tag="nwr")
                nc.vector.tensor_scalar(t0[:], v_ap, 0.5, 0.5, Alu.mult, Alu.add)
                nc.vector.reciprocal(r[:], t0[:])
                for it in range(iters):
                    dst = out_ap if it == iters - 1 else r[:]
                    nc.vector.tensor_tensor(t0[:], r[:], r[:], Alu.mult)
                    nc.vector.scalar_tensor_tensor(t1[:], t0[:], -0.5, v_ap,
                                                   Alu.mult, Alu.mult)
                    nc.vector.scalar_tensor_tensor(dst, t1[:], 1.5, r[:],
                                                   Alu.add, Alu.mult)

            def phaseB_stats(s):
                """mean/std from Mx/Mx2 + mean-half of the bias matvec."""
                d = st[s]
                meanc = smpool.tile([128, NCH], f32, name="meanc", tag="meanc")
                nc.vector.tensor_scalar(meanc[:], d["Mx"][:], 1.0 / T, None, Alu.mult)
                pmv = pmvp.tile([A, 1], f32, name="pmv", tag="pmv")
                d["pmv"] = pmv
                for k in range(NCH):
                    nc.tensor.matmul(pmv[:], wms_t[k], meanc[:, k:k + 1],
                                     start=(k == 0), stop=False,
                                     skip_group_check=True)
                # unbiased var = (Mx2 - T*mean^2) / (T-1);  T*mean^2 = mean*Mx
                tm2 = smpool.tile([128, NCH], f32, name="tm2", tag="tm2")
                nc.vector.tensor_tensor(tm2[:], meanc[:], d["Mx"][:], Alu.mult)
                vdiff = smpool.tile([128, NCH], f32, name="vdiff", tag="vdiff")
                nc.vector.scalar_tensor_tensor(vdiff[:], tm2[:], -1.0, d["Mx2"][:],
                                               Alu.mult, Alu.add)
                v = smpool.tile([128, NCH], f32, name="v", tag="v")
                nc.vector.tensor_scalar(v[:], vdiff[:], 1.0 / (T - 1.0), CLAMP,
                                        Alu.mult, Alu.max)
                std_t = smpool.tile([128, NCH], f32, name="std_t", tag="std_t")
                d["std_t"] = std_t
                rs = smpool.tile([128, NCH], f32, name="rs", tag="rs")
                newton_rsqrt(v[:], rs[:], NCH, 2)
                nc.vector.tensor_tensor(std_t[:], v[:], rs[:], Alu.mult)

            def phaseB_matvec2(s):
                """std-half of the matvec + btot."""
                d = st[s]
                pmv, std_t = d["pmv"], d["std_t"]
                for k in range(NCH):
                    nc.tensor.matmul(pmv[:], wms_t[NCH + k], std_t[:, k:k + 1],
                                     start=False, stop=(k == NCH - 1),
                                     skip_group_check=True)
                btot = smpool.tile([A, 1], f32, name="btot", tag="btot")
                nc.vector.tensor_tensor(btot[:], pmv[:], b1_t[:], Alu.add)
                d["btot"] = btot

            def phaseB_relu(s):
                d = st[s]
                rt = rpool.tile([A, T], bf16, name="r", tag="r")
                nc.scalar.activation(rt[:], d["ph1"][:], Act.Relu, bias=d["btot"][:])
                d["rt"] = rt

            def phaseB_tanh(s):
                d = st[s]
                gt = gpool.tile([A, T], bf16, name="g", tag="g")
                nc.scalar.activation(gt[:], d["rt"][:], Act.Tanh, bias=add1_t[:],
                                     scale=inv1_t[:])
                d["g"] = gt

            def phaseB(s):
                phaseB_stats(s)
                phaseB_matvec2(s)
                phaseB_relu(s)
                phaseB_tanh(s)

            def phaseC_mm2exp(s, c):
                """PE matmul2 + ACT exp for chunk c (emitted one chunk ahead
                of the DVE consumer so the in-order ACT stream never starves
                the DVE)."""
                d = st[s]
                if c == 0:
                    d["S0"] = spool.tile([128, NCH], f32, name="S0", tag="S0")
                    d["S1"] = spool.tile([128, NCH], f32, name="S1", tag="S1")
                    d["S2"] = spool.tile([128, NCH], f32, name="S2", tag="S2")
                    d["E"] = [None] * NCH
                p2 = p2p.tile([128, T], f32, name="p2", tag="p2")
                wsl = w2T_t[:, c * 128:(c + 1) * 128]
                for lo, hi in HALVES:
                    nc.tensor.matmul(p2[:, lo:hi], wsl, d["g"][:, lo:hi],
                                     start=True, stop=True)
                E = epool.tile([128, T], bf16, name="E", tag="E")
                nc.scalar.activation(E[:], p2[:], Act.Exp,
                                     bias=b2p_t[:, c:c + 1], scale=inv2_t[:, c:c + 1])
                d["E"][c] = E

            def phaseC_dve(s, c, qt_pool=True, s1_act=False):
                """eb/pt/S1 for chunk c; qt on Pool (or DVE); S2 of chunk c-2
                (deferred two chunks so the DVE never waits on Pool's qt)."""
                d = st[s]
                E = d["E"][c]
                d["E"][c] = None
                eb = ebpool.tile([128, T], bf16, name="eb", tag="eb")
                nc.vector.tensor_scalar(eb[:], E[:], 1.0, 0.0, Alu.max, Alu.add,
                                        accum_out=d["S0"][:, c:c + 1])
                xt = d["x"][c]
                pt = ppool.tile([128, T], bf16, name="p", tag="p")
                nc.vector.tensor_tensor(pt[:], eb[:], xt, Alu.mult)
                qt = qpool.tile([128, T], bf16, name="q", tag="q")
                if qt_pool:
                    nc.gpsimd.tensor_tensor(qt[:], pt[:], xt, Alu.mult)
                else:
                    nc.vector.tensor_tensor(qt[:], pt[:], xt, Alu.mult)
                d["qt_%d" % c] = qt
                if s1_act:
                    j1 = jpool.tile([128, T], bf16, name="junk", tag="junk")
                    nc.scalar.activation(j1[:], pt[:], Act.Identity,
                                         accum_out=d["S1"][:, c:c + 1])
                else:
                    j1 = jpool.tile([128, T], bf16, name="junk", tag="junk")
                    nc.vector.tensor_scalar(j1[:], pt[:], 0.0, 0.0, Alu.add,
                                            Alu.add,
                                            accum_out=d["S1"][:, c:c + 1])

            def phaseC_s2(s, c):
                """S2 accumulate for chunk c (reads qt produced on Pool)."""
                d = st[s]
                qt = d.pop("qt_%d" % c)
                j2 = jpool.tile([128, T], bf16, name="junk", tag="junk")
                nc.vector.tensor_scalar(j2[:], qt[:], 0.0, 0.0, Alu.add, Alu.add,
                                        accum_out=d["S2"][:, c:c + 1])

            def store_half(s, half, srct):
                ptr = ptrp.tile([NCH, 128], f32, name="ptr", tag="ptr")
                nc.tensor.transpose(ptr[:], srct[:], ident_t[:])
                ost = opool.tile([NCH, 128], f32, name="ost", tag="ost")
                nc.scalar.copy(ost[:], ptr[:])
                dst = out[s, half * C:(half + 1) * C]
                dst = dst.rearrange("(ci p) -> ci p", p=128)
                nc.sync.dma_start(dst, ost[:])

            def sample_out_mu(s):
                """mu (needs only S0/S1) + transpose (PE) + store."""
                d = st[s]
                rc = smpool.tile([128, NCH], f32, name="rc", tag="rc")
                nc.vector.reciprocal(rc[:], d["S0"][:])
                d["rc"] = rc
                mu = opool.tile([128, NCH], f32, name="mu", tag="mu")
                nc.vector.tensor_tensor(mu[:], d["S1"][:], rc[:], Alu.mult)
                d["mu"] = mu
                store_half(s, 0, mu)

            def sample_out_sg(s):
                """sg (needs S2) + transpose (PE) + store."""
                d = st[s]
                rc, mu = d["rc"], d["mu"]
                sg = opool.tile([128, NCH], f32, name="sg", tag="sg")
                ex2 = smpool.tile([128, NCH], f32, name="ex2", tag="ex2")
                nc.vector.tensor_tensor(ex2[:], d["S2"][:], rc[:], Alu.mult)
                mu2 = smpool.tile([128, NCH], f32, name="mu2", tag="mu2")
                nc.vector.tensor_tensor(mu2[:], mu[:], mu[:], Alu.mult)
                sg2 = smpool.tile([128, NCH], f32, name="sg2", tag="sg2")
                nc.vector.scalar_tensor_tensor(sg2[:], mu2[:], -1.0, ex2[:],
                                               Alu.mult, Alu.add)
                v2 = smpool.tile([128, NCH], f32, name="v2", tag="v2")
                nc.vector.tensor_scalar(v2[:], sg2[:], 1.0, CLAMP, Alu.mult, Alu.max)
                rsg = smpool.tile([128, NCH], f32, name="rsg", tag="rsg")
                newton_rsqrt(v2[:], rsg[:], NCH, 2)
                nc.vector.tensor_tensor(sg[:], v2[:], rsg[:], Alu.mult)
                store_half(s, 1, sg)

            # ---------------- constant loads (interleaved with x below) ----
            def load_w1xT():
                t = cpool.tile([128, NCH * A], bf16, name="w1xall", tag="w1xall")
                src_ap = w1xT.rearrange("(c p) a -> p c a", p=128)
                nc.sync.dma_start(t[:].rearrange("p (c a) -> p c a", a=A), src_ap)
                return [t[:, c * A:(c + 1) * A] for c in range(NCH)]

            def load_params():
                global b1_t, inv1_t, add1_t, inv2_t, b2p_t, w2T_t, wms_t, ident_t
                b1_t = cpool.tile([A, 1], f32, name="b1", tag="b1")
                nc.sync.dma_start(b1_t[:], b1d[:])
                inv1_t = cpool.tile([A, 1], f32, name="inv1", tag="inv1")
                nc.sync.dma_start(inv1_t[:], inv1d[:])
                add1_t = cpool.tile([A, 1], f32, name="add1", tag="add1")
                nc.sync.dma_start(add1_t[:], add1d[:])
                inv2_t = cpool.tile([128, NCH], f32, name="inv2", tag="inv2")
                nc.sync.dma_start(inv2_t[:], inv2d[:])
                b2p_t = cpool.tile([128, NCH], f32, name="b2p", tag="b2p")
                nc.sync.dma_start(b2p_t[:], b2pd[:])
                w2T_t = cpool.tile([A, C], bf16, name="w2T", tag="w2T")
                nc.sync.dma_start(w2T_t[:], w2T[:])
                ident_t = cpool.tile([128, 128], f32, name="ident", tag="ident")
                nc.sync.dma_start(ident_t[:], identd[:])
                wt = cpool.tile([128, 2 * NCH * A], f32, name="wmsall", tag="wmsall")
                src_ap = wmsT.rearrange("(k p) a -> p k a", p=128)
                nc.sync.dma_start(wt[:].rearrange("p (k a) -> p k a", a=A), src_ap)
                wms_t = [wt[:, k * A:(k + 1) * A] for k in range(2 * NCH)]

            def body():
                global w1xT_t
                # prologue: phase A of samples 0/1, weights interleaved,
                # sample 2's DMA prefetched.  sum(x^2) rotates over
                # DVE/ACT/Pool so no engine serializes the (un-overlapped)
                # prologue.
                dma_x(0, groups=[0], split_first=True)
                w1xT_t = load_w1xT()
                dma_x(0, groups=[1, 2, 3])
                for c in range(NCH):
                    phaseA_moments(0, c, sq=PRO_SQ[c])
                    phaseA_mm1(0, c)
                load_params()
                dma_x(1)
                for c in range(NCH):
                    phaseA_moments(1, c, sq=PRO_SQ[(c + 6) % NCH])
                dma_x(2)
                phaseB(0)
                for c in range(NCH):
                    phaseA_mm1(1, c)
                # steady state: C(s) carries A(s+2) moments, B(s+1) spread
                # over c3/c4/c5/c8, and A(s+2)'s matmul1 in the c>=8 shadow
                # of relu(s+1) freeing the ph1 slot.  ACT's exp runs one
                # chunk ahead of the DVE; S2 accumulation runs one chunk
                # behind (its qt comes from the Pool engine).
                # (s, c) pairs for the exp-lookahead stream, two chunks ahead
                # of the DVE consumer.  g(s) is ready by c==8 of C(s-1), so
                # (s+1, 0) may be emitted from c==10 of C(s) onward.
                mm2exp_seq = [(s, c) for s in range(SPC) for c in range(NCH)]
                mm2exp_pos = 0

                def emit_mm2exp_upto(i):
                    nonlocal mm2exp_pos
                    while mm2exp_pos <= i and mm2exp_pos < len(mm2exp_seq):
                        phaseC_mm2exp(*mm2exp_seq[mm2exp_pos])
                        mm2exp_pos += 1

                emit_mm2exp_upto(1)
                for s in range(SPC):
                    has_a = s + 2 < SPC          # phase-A work interleaved?
                    qt_map = FRONT_QT if has_a else TAIL_QT
                    if s == SPC - 1:
                        qt_map = qt_map[:NCH - 2] + "DD"
                    s1_map = FRONT_S1 if has_a else TAIL_S1
                    for c in range(NCH):
                        emit_mm2exp_upto(s * NCH + c + 2)
                        phaseC_dve(s, c, qt_pool=(qt_map[c] == "P"),
                                   s1_act=(s1_map[c] == "A"))
                        if c > 1:
                            phaseC_s2(s, c - 2)
                        # deferred flush of the previous sample: its last S2s
                        # and the whole output stage run here, where the
                        # Pool's qt and the serial small-op chain can hide
                        # under this sample's chunk stream.
                        if c == 0 and s - 1 in st:
                            phaseC_s2(s - 1, NCH - 2)
                            sample_out_mu(s - 1)
                        if c == 1 and s - 1 in st:
                            phaseC_s2(s - 1, NCH - 1)
                        if c == 2 and s - 1 in st:
                            sample_out_sg(s - 1)
                            del st[s - 1]
                        if s + 3 < SPC and c == 0:
                            dma_x(s + 3)
                        if s == 2 and c < 4:
                            phaseA_moments(3, 8 + c, sq=FRONT_SQ[8 + c])
                        if has_a:
                            if s == 0:
                                # x(2)'s DMA was only issued in the prologue:
                                # shift A(2) two chunks later so the in-order
                                # DVE stream doesn't park on it.
                                if c >= 2:
                                    phaseA_moments(2, c - 2, sq=FRONT_SQ[c - 2])
                            elif s == 1:
                                # A(3) spills 4 chunks into C(2) to even the
                                # front/tail engine loads.
                                if c < 8:
                                    phaseA_moments(3, c, sq=FRONT_SQ[c])
                                if c < 2:
                                    phaseA_moments(2, 10 + c, sq=FRONT_SQ[10 + c])
                            else:
                                phaseA_moments(s + 2, c, sq=FRONT_SQ[c])
                        bc = (5, 6, 7, 8) if s == 2 else (3, 4, 5, 8)
                        if c == bc[0] and s + 1 < SPC:
                            phaseB_stats(s + 1)
                        if c == bc[1] and s + 1 < SPC:
                            phaseB_matvec2(s + 1)
                        if c == bc[2] and s + 1 < SPC:
                            phaseB_relu(s + 1)
                        if c == bc[3] and s + 1 < SPC:
                            phaseB_tanh(s + 1)
                        # mm1(s+2) reuses the single ph1 bank; its first
                        # matmul must be emitted after relu(s+1) has read it.
                        if has_a and c >= 8:
                            for cc in range(3 * (c - 8), 3 * (c - 8) + 3):
                                phaseA_mm1(s + 2, cc)
                s = SPC - 1
                phaseC_s2(s, NCH - 2)
                sample_out_mu(s)
                phaseC_s2(s, NCH - 1)
                sample_out_sg(s)
                del st[s]

            if loop_reps == 1:
                body()
            else:
                with tc.For_i(0, loop_reps, 1):
                    body()

    nc.compile()
    return nc


def _get_module(loop_reps=1):
    key = loop_reps
    if key not in _CACHE:
        _CACHE[key] = _build_module(loop_reps)
    return _CACHE[key]


def _host_prep(inputs):
    """Precompute folded parameters and shard inputs. Returns per-core in_maps."""
    x = np.asarray(inputs["x"])
    W1 = np.asarray(inputs["W1"], np.float32)
    b1 = np.asarray(inputs["b1"], np.float32)
    g1 = np.asarray(inputs["g1"], np.float32)
    beta1 = np.asarray(inputs["beta1"], np.float32)
    rm1 = np.asarray(inputs["rm1"], np.float32)
    rv1 = np.asarray(inputs["rv1"], np.float32)
    W2 = np.asarray(inputs["W2"], np.float32)
    b2 = np.asarray(inputs["b2"], np.float32)
    g2 = np.asarray(inputs["g2"], np.float32)
    rv2 = np.asarray(inputs["rv2"], np.float32)

    inv1 = (g1 / np.sqrt(rv1 + BN_EPS)).astype(np.float32)
    add1 = (beta1 - rm1 * inv1).astype(np.float32)
    inv2 = (g2 / np.sqrt(rv2 + BN_EPS)).astype(np.float32)
    b2p = (inv2 * b2).astype(np.float32)

    const = {
        "w1xT": np.ascontiguousarray(W1[:, :C].T).astype(ml_dtypes.bfloat16),
        "wmsT": np.ascontiguousarray(W1[:, C:].T).astype(np.float32),
        "w2T": np.ascontiguousarray(W2.T).astype(ml_dtypes.bfloat16),
        "b1d": b1.reshape(A, 1),
        "inv1d": inv1.reshape(A, 1),
        "add1d": add1.reshape(A, 1),
        "inv2d": np.ascontiguousarray(inv2.reshape(NCH, 128).T),
        "b2pd": np.ascontiguousarray(b2p.reshape(NCH, 128).T),
        "identd": np.eye(128, dtype=np.float32),
    }
    xbf = x.astype(ml_dtypes.bfloat16)
    in_maps = []
    for core in range(N_CORES):
        m = dict(const)
        m["xbf"] = np.ascontiguousarray(xbf[core * SPC:(core + 1) * SPC])
        in_maps.append(m)
    return in_maps


def kernel(**inputs):
    from concourse.bass_utils import run_bass_kernel_spmd

    nc = _get_module(loop_reps=1)
    in_maps = _host_prep(inputs)
    res = run_bass_kernel_spmd(nc, in_maps, core_ids=list(range(N_CORES)))
    out = np.concatenate([res.results[i]["out"] for i in range(N_CORES)], axis=0)
    return out.astype(np.float32)


# revision 44
# speedup vs baseline: 1257.9178x; 1.0038x over previous
"""AttentiveStatPooling Trainium2 kernel (8-core SPMD, data-parallel over batch).

Contract: kernel(**inputs) takes the FULL unsharded inputs (as produced by
reference.setup_inputs()) and returns the FULL [B, 2C] output.

Math (per sample, identical to the jax reference):
  mean/std over T of x;  h = relu(Wx@x + (Wm@mean + Ws@std + b1));
  g = tanh(BN1(h));  l = BN2scale * relu(W2@g + b2)  (the BN2 shift cancels in
  the softmax and is dropped);  w = softmax(l, axis=T);
  out = [sum(x*w), sqrt(clip(sum(x^2*w) - mu^2, 1e-4))].

Implementation notes (v3 — four-engine balance):
  - batch 32 split 4 samples/core across 8 NeuronCores (pure DP).
  - x shipped in bf16 (halves DMA; all elementwise work runs in bf16 so the
    DVE hits its 2x (tensor_tensor) / 4x (tensor_scalar) perf modes).
  - per-chunk softmax stats with fused accumulates (no standalone reduce):
      eb = max(E,1)  tensor_scalar(max) 4x, fused accum -> S0   [DVE]
      pt = eb*x      tensor_tensor 2x                           [DVE]
      S1 = sum(pt)   tensor_scalar(id) 4x + accum               [DVE or ACT]
      qt = pt*x      tensor_tensor                              [Pool or DVE]
      S2 = sum(qt)   tensor_scalar(id) 4x + accum               [DVE]
    The qt multiply rides the otherwise-idle GPSIMD/Pool engine (plain
    tensor_tensor is the only elementwise op walrus allows there); its S2
    accumulate is deferred two chunks so the DVE never waits on the Pool.
  - x moments per chunk: sum(x) on DVE (tensor_scalar 4x + accum),
    sum(x^2) on ACT (Square+accum) / DVE (x*x + accum) per the *_SQ maps.
    All ACT functions used (Exp/Tanh/Square/Relu/Identity) live in one
    activation table set, so there are no table reloads.
  - softmax needs no max-subtraction (logits bounded, per-row shift cancels);
    relu inside the softmax realized as max(exp(l), 1).
  - sqrt via Newton/rsqrt on the vector engine (avoids ACT table switches),
    3 DVE ops per iteration via scalar_tensor_tensor.
  - emission is software-pipelined: mm2+exp run two chunks ahead of the DVE
    consumer, phase A of sample s+2 and phase B of sample s+1 interleave
    into phase C of sample s, and the previous sample's S2 flush + output
    stage defer into the next sample's chunk stream.  The FRONT_*/TAIL_*/PRO_* knob
    strings balance per-chunk work across DVE/ACT/Pool for the "front"
    samples (which carry phase-A work) and the "tail" samples (which don't).
"""

import numpy as np
import ml_dtypes

B, C, T, A = 32, 1536, 1000, 128
N_CORES = 8
SPC = B // N_CORES        # samples per core
NCH = C // 128            # 12 channel chunks of 128
BN_EPS = 1e-5
CLAMP = 1e-4
HALVES = ((0, 512), (512, 1000))   # psum-bank-aligned split of T

_CACHE = {}

# Engine-balance knobs (index = c % 12).  For each chunk of "front" samples
# (those that also carry phase-A work of sample s+2) and "tail" samples:
#   qt engine: 'P' (Pool) or 'D' (DVE)
#   S1 accumulate: 'D' (DVE tensor_scalar) or 'A' (ACT identity+accum)
#   phase-A sum(x^2): 'A' (ACT Square), 'D' (DVE x*x+accum), 'P' (Pool x*x
#   + DVE accum)
FRONT_QT = "PPPPPPPPPPPP"
FRONT_S1 = "DDDDDDDDDDDD"
FRONT_SQ = "AAADAAADAAAD"
TAIL_QT = "DPPPDPPPDPPP"
TAIL_S1 = "ADADDADADDAD"
PRO_SQ = "DADADADADADA"
FRONT_MX = "DDDDDDDDDDDD"   # prologue samples 0/1 (rotated by 6 for s=1)


def _build_module(loop_reps=1):
    import concourse.tile as tile
    from concourse import bacc, mybir
    from contextlib import ExitStack

    f32, bf16 = mybir.dt.float32, mybir.dt.bfloat16
    Alu = mybir.AluOpType
    Act = mybir.ActivationFunctionType

    nc = bacc.Bacc("TRN2", target_bir_lowering=False, debug=False,
                   num_devices=N_CORES)

    xbf = nc.dram_tensor("xbf", [SPC, C, T], bf16, kind="ExternalInput").ap()
    w1xT = nc.dram_tensor("w1xT", [C, A], bf16, kind="ExternalInput").ap()
    wmsT = nc.dram_tensor("wmsT", [2 * C, A], f32, kind="ExternalInput").ap()
    w2T = nc.dram_tensor("w2T", [A, C], bf16, kind="ExternalInput").ap()
    b1d = nc.dram_tensor("b1d", [A, 1], f32, kind="ExternalInput").ap()
    inv1d = nc.dram_tensor("inv1d", [A, 1], f32, kind="ExternalInput").ap()
    add1d = nc.dram_tensor("add1d", [A, 1], f32, kind="ExternalInput").ap()
    inv2d = nc.dram_tensor("inv2d", [128, NCH], f32, kind="ExternalInput").ap()
    b2pd = nc.dram_tensor("b2pd", [128, NCH], f32, kind="ExternalInput").ap()
    identd = nc.dram_tensor("identd", [128, 128], f32, kind="ExternalInput").ap()
    out = nc.dram_tensor("out", [SPC, 2 * C], f32, kind="ExternalOutput").ap()

    with tile.TileContext(nc) as tc:
        with ExitStack() as ctx:
            cpool = ctx.enter_context(tc.tile_pool(name="const", bufs=1))
            xpool = ctx.enter_context(tc.tile_pool(name="x", bufs=14))
            epool = ctx.enter_context(tc.tile_pool(name="e", bufs=3))
            ebpool = ctx.enter_context(tc.tile_pool(name="eb", bufs=3))
            ppool = ctx.enter_context(tc.tile_pool(name="p", bufs=3))
            qpool = ctx.enter_context(tc.tile_pool(name="q", bufs=4))
            jpool = ctx.enter_context(tc.tile_pool(name="junk", bufs=6))
            rpool = ctx.enter_context(tc.tile_pool(name="r", bufs=2))
            gpool = ctx.enter_context(tc.tile_pool(name="g", bufs=2))
            spool = ctx.enter_context(tc.tile_pool(name="stats", bufs=4))
            smpool = ctx.enter_context(tc.tile_pool(name="small", bufs=8))
            opool = ctx.enter_context(tc.tile_pool(name="ostage", bufs=4))
            ph1p = ctx.enter_context(tc.tile_pool(name="ph1", bufs=1, space="PSUM"))
            p2p = ctx.enter_context(tc.tile_pool(name="p2", bufs=2, space="PSUM"))
            pmvp = ctx.enter_context(tc.tile_pool(name="pmv", bufs=1, space="PSUM"))
            ptrp = ctx.enter_context(tc.tile_pool(name="ptr", bufs=1, space="PSUM"))

            st = {}   # per-sample state

            def dma_x(s, groups=range(4), split_first=False):
                if s not in st:
                    st[s] = {"xg": [], "x": []}
                for g in groups:
                    xt = xpool.tile([128, 3 * T], bf16, name="x", tag="x")
                    src_ap = xbf[s, g * 384:(g + 1) * 384, :]
                    src_ap = src_ap.rearrange("(c p) t -> p c t", p=128)
                    dst = xt[:].rearrange("p (c t) -> p c t", t=T)
                    if split_first and g == 0:
                        # land chunk 0 first so phase A can start sooner
                        nc.sync.dma_start(dst[:, 0:1], src_ap[:, 0:1])
                        nc.sync.dma_start(dst[:, 1:3], src_ap[:, 1:3])
                    else:
                        nc.sync.dma_start(dst, src_ap)
                    st[s]["xg"].append(xt)
                    for i in range(3):
                        st[s]["x"].append(xt[:, i * T:(i + 1) * T])

            def phaseA_moments(s, c, sq="A"):
                """sum(x) on DVE (4x) + sum(x^2) on ACT (Square+accum), DVE
                (x*x then accum), or Pool (x*x) + DVE accum."""
                d = st[s]
                if c == 0:
                    d["Mx"] = spool.tile([128, NCH], f32, name="Mx", tag="Mx")
                    d["Mx2"] = spool.tile([128, NCH], f32, name="Mx2", tag="Mx2")
                xt = d["x"][c]
                j0 = jpool.tile([128, T], bf16, name="junk", tag="junk")
                if FRONT_MX[c] == "A":
                    nc.scalar.activation(j0[:], xt, Act.Identity,
                                         accum_out=d["Mx"][:, c:c + 1])
                else:
                    nc.vector.tensor_scalar(j0[:], xt, 0.0, 0.0, Alu.add,
                                            Alu.add,
                                            accum_out=d["Mx"][:, c:c + 1])
                if sq == "A":
                    j1 = jpool.tile([128, T], bf16, name="junk", tag="junk")
                    nc.scalar.activation(j1[:], xt, Act.Square,
                                         accum_out=d["Mx2"][:, c:c + 1])
                else:
                    x2 = jpool.tile([128, T], bf16, name="junk", tag="junk")
                    if sq == "P":
                        nc.gpsimd.tensor_tensor(x2[:], xt, xt, Alu.mult)
                    else:
                        nc.vector.tensor_tensor(x2[:], xt, xt, Alu.mult)
                    j1 = jpool.tile([128, T], bf16, name="junk", tag="junk")
                    nc.vector.tensor_scalar(j1[:], x2[:], 0.0, 0.0, Alu.add,
                                            Alu.add,
                                            accum_out=d["Mx2"][:, c:c + 1])

            def phaseA_mm1(s, c):
                d = st[s]
                if c == 0:
                    d["ph1"] = ph1p.tile([A, T], f32, name="ph1", tag="ph1")
                xt = d["x"][c]
                for lo, hi in HALVES:
                    nc.tensor.matmul(d["ph1"][:, lo:hi], w1xT_t[c],
                                     xt[:, lo:hi], start=(c == 0),
                                     stop=(c == NCH - 1), skip_group_check=True)

            def newton_rsqrt(v_ap, out_ap, n, iters):
                """out = 1/sqrt(v) elementwise on a [128, n] fp32 AP.
                Seed r0 = 2/(1+v) (robust for any v>0), then Newton
                iterations r' = r*(1.5 - 0.5*v*r^2), 3 DVE ops each:
                  t = r*r;  u = (t*-0.5)*v;  r' = (u+1.5)*r."""
                t0 = smpool.tile([128, n], f32, name="nw0", tag="nw0")
                t1 = smpool.tile([128, n], f32, name="nw1", tag="nw1")
                r = smpool.tile([128, n], f32, name="nwr", tag="nwr")
                nc.vector.tensor_scalar(t0[:], v_ap, 0.5, 0.5, Alu.mult, Alu.add)
                nc.vector.reciprocal(r[:], t0[:])
                for it in range(iters):
                    dst = out_ap if it == iters - 1 else r[:]
                    nc.vector.tensor_tensor(t0[:], r[:], r[:], Alu.mult)
                    nc.vector.scalar_tensor_tensor(t1[:], t0[:], -0.5, v_ap,
                                                   Alu.mult, Alu.mult)
                    nc.vector.scalar_tensor_tensor(dst, t1[:], 1.5, r[:],
                                                   Alu.add, Alu.mult)

            def phaseB_stats(s):
                """mean/std from Mx/Mx2 + mean-half of the bias matvec."""
                d = st[s]
                meanc = smpool.tile([128, NCH], f32, name="meanc", tag="meanc")
                nc.vector.tensor_scalar(meanc[:], d["Mx"][:], 1.0 / T, None, Alu.mult)
                pmv = pmvp.tile([A, 1], f32, name="pmv", tag="pmv")
                d["pmv"] = pmv
                for k in range(NCH):
                    nc.tensor.matmul(pmv[:], wms_t[k], meanc[:, k:k + 1],
                                     start=(k == 0), stop=False,
                                     skip_group_check=True)
                # unbiased var = (Mx2 - T*mean^2) / (T-1);  T*mean^2 = mean*Mx
                tm2 = smpool.tile([128, NCH], f32, name="tm2", tag="tm2")
                nc.vector.tensor_tensor(tm2[:], meanc[:], d["Mx"][:], Alu.mult)
                vdiff = smpool.tile([128, NCH], f32, name="vdiff", tag="vdiff")
                nc.vector.scalar_tensor_tensor(vdiff[:], tm2[:], -1.0, d["Mx2"][:],
                                               Alu.mult, Alu.add)
                v = smpool.tile([128, NCH], f32, name="v", tag="v")
                nc.vector.tensor_scalar(v[:], vdiff[:], 1.0 / (T - 1.0), CLAMP,
                                        Alu.mult, Alu.max)
                std_t = smpool.tile([128, NCH], f32, name="std_t", tag="std_t")
                d["std_t"] = std_t
                rs = smpool.tile([128, NCH], f32, name="rs", tag="rs")
                newton_rsqrt(v[:], rs[:], NCH, 2)
                nc.vector.tensor_tensor(std_t[:], v[:], rs[:], Alu.mult)

            def phaseB_matvec2(s):
                """std-half of the matvec + btot."""
                d = st[s]
                pmv, std_t = d["pmv"], d["std_t"]
                for k in range(NCH):
                    nc.tensor.matmul(pmv[:], wms_t[NCH + k], std_t[:, k:k + 1],
                                     start=False, stop=(k == NCH - 1),
                                     skip_group_check=True)
                btot = smpool.tile([A, 1], f32, name="btot", tag="btot")
                nc.vector.tensor_tensor(btot[:], pmv[:], b1_t[:], Alu.add)
                d["btot"] = btot

            def phaseB_relu(s):
                d = st[s]
                rt = rpool.tile([A, T], bf16, name="r", tag="r")
                nc.scalar.activation(rt[:], d["ph1"][:], Act.Relu, bias=d["btot"][:])
                d["rt"] = rt

            def phaseB_tanh(s):
                d = st[s]
                gt = gpool.tile([A, T], bf16, name="g", tag="g")
                nc.scalar.activation(gt[:], d["rt"][:], Act.Tanh, bias=add1_t[:],
                                     scale=inv1_t[:])
                d["g"] = gt

            def phaseB(s):
                phaseB_stats(s)
                phaseB_matvec2(s)
                phaseB_relu(s)
                phaseB_tanh(s)

            def phaseC_mm2exp(s, c):
                """PE matmul2 + ACT exp for chunk c (emitted one chunk ahead
                of the DVE consumer so the in-order ACT stream never starves
                the DVE)."""
                d = st[s]
                if c == 0:
                    d["S0"] = spool.tile([128, NCH], f32, name="S0", tag="S0")
                    d["S1"] = spool.tile([128, NCH], f32, name="S1", tag="S1")
                    d["S2"] = spool.tile([128, NCH], f32, name="S2", tag="S2")
                    d["E"] = [None] * NCH
                p2 = p2p.tile([128, T], f32, name="p2", tag="p2")
                wsl = w2T_t[:, c * 128:(c + 1) * 128]
                for lo, hi in HALVES:
                    nc.tensor.matmul(p2[:, lo:hi], wsl, d["g"][:, lo:hi],
                                     start=True, stop=True)
                E = epool.tile([128, T], bf16, name="E", tag="E")
                nc.scalar.activation(E[:], p2[:], Act.Exp,
                                     bias=b2p_t[:, c:c + 1], scale=inv2_t[:, c:c + 1])
                d["E"][c] = E

            def phaseC_dve(s, c, qt_pool=True, s1_act=False):
                """eb/pt/S1 for chunk c; qt on Pool (or DVE); S2 of chunk c-2
                (deferred two chunks so the DVE never waits on Pool's qt)."""
                d = st[s]
                E = d["E"][c]
                d["E"][c] = None
                eb = ebpool.tile([128, T], bf16, name="eb", tag="eb")
                nc.vector.tensor_scalar(eb[:], E[:], 1.0, 0.0, Alu.max, Alu.add,
                                        accum_out=d["S0"][:, c:c + 1])
                xt = d["x"][c]
                pt = ppool.tile([128, T], bf16, name="p", tag="p")
                nc.vector.tensor_tensor(pt[:], eb[:], xt, Alu.mult)
                qt = qpool.tile([128, T], bf16, name="q", tag="q")
                if qt_pool:
                    nc.gpsimd.tensor_tensor(qt[:], pt[:], xt, Alu.mult)
                else:
                    nc.vector.tensor_tensor(qt[:], pt[:], xt, Alu.mult)
                d["qt_%d" % c] = qt
                if s1_act:
                    j1 = jpool.tile([128, T], bf16, name="junk", tag="junk")
                    nc.scalar.activation(j1[:], pt[:], Act.Identity,
                                         accum_out=d["S1"][:, c:c + 1])
                else:
                    j1 = jpool.tile([128, T], bf16, name="junk", tag="junk")
                    nc.vector.tensor_scalar(j1[:], pt[:], 0.0, 0.0, Alu.add,
                                            Alu.add,
                                            accum_out=d["S1"][:, c:c + 1])

            def phaseC_s2(s, c):
                """S2 accumulate for chunk c (reads qt produced on Pool)."""
                d = st[s]
                qt = d.pop("qt_%d" % c)
                j2 = jpool.tile([128, T], bf16, name="junk", tag="junk")
                nc.vector.tensor_scalar(j2[:], qt[:], 0.0, 0.0, Alu.add, Alu.add,
                                        accum_out=d["S2"][:, c:c + 1])

            def store_half(s, half, srct):
                ptr = ptrp.tile([NCH, 128], f32, name="ptr", tag="ptr")
                nc.tensor.transpose(ptr[:], srct[:], ident_t[:])
                ost = opool.tile([NCH, 128], f32, name="ost", tag="ost")
                nc.scalar.copy(ost[:], ptr[:])
                dst = out[s, half * C:(half + 1) * C]
                dst = dst.rearrange("(ci p) -> ci p", p=128)
                nc.sync.dma_start(dst, ost[:])

            def sample_out_mu(s):
                """mu (needs only S0/S1) + transpose (PE) + store."""
                d = st[s]
                rc = smpool.tile([128, NCH], f32, name="rc", tag="rc")
                nc.vector.reciprocal(rc[:], d["S0"][:])
                d["rc"] = rc
                mu = opool.tile([128, NCH], f32, name="mu", tag="mu")
                nc.vector.tensor_tensor(mu[:], d["S1"][:], rc[:], Alu.mult)
                d["mu"] = mu
                store_half(s, 0, mu)

            def sample_out_sg(s, iters=2):
                """sg (needs S2) + transpose (PE) + store."""
                d = st[s]
                rc, mu = d["rc"], d["mu"]
                sg = opool.tile([128, NCH], f32, name="sg", tag="sg")
                ex2 = smpool.tile([128, NCH], f32, name="ex2", tag="ex2")
                nc.vector.tensor_tensor(ex2[:], d["S2"][:], rc[:], Alu.mult)
                mu2 = smpool.tile([128, NCH], f32, name="mu2", tag="mu2")
                nc.vector.tensor_tensor(mu2[:], mu[:], mu[:], Alu.mult)
                sg2 = smpool.tile([128, NCH], f32, name="sg2", tag="sg2")
                nc.vector.scalar_tensor_tensor(sg2[:], mu2[:], -1.0, ex2[:],
                                               Alu.mult, Alu.add)
                v2 = smpool.tile([128, NCH], f32, name="v2", tag="v2")
                nc.vector.tensor_scalar(v2[:], sg2[:], 1.0, CLAMP, Alu.mult, Alu.max)
                rsg = smpool.tile([128, NCH], f32, name="rsg", tag="rsg")
                newton_rsqrt(v2[:], rsg[:], NCH, iters)
                nc.vector.tensor_tensor(sg[:], v2[:], rsg[:], Alu.mult)
                store_half(s, 1, sg)

            # ---------------- constant loads (interleaved with x below) ----
            def load_w1xT():
                t = cpool.tile([128, NCH * A], bf16, name="w1xall", tag="w1xall")
                src_ap = w1xT.rearrange("(c p) a -> p c a", p=128)
                nc.sync.dma_start(t[:].rearrange("p (c a) -> p c a", a=A), src_ap)
                return [t[:, c * A:(c + 1) * A] for c in range(NCH)]

            def load_params():
                global b1_t, inv1_t, add1_t, inv2_t, b2p_t, w2T_t, wms_t, ident_t
                b1_t = cpool.tile([A, 1], f32, name="b1", tag="b1")
                nc.sync.dma_start(b1_t[:], b1d[:])
                inv1_t = cpool.tile([A, 1], f32, name="inv1", tag="inv1")
                nc.sync.dma_start(inv1_t[:], inv1d[:])
                add1_t = cpool.tile([A, 1], f32, name="add1", tag="add1")
                nc.sync.dma_start(add1_t[:], add1d[:])
                inv2_t = cpool.tile([128, NCH], f32, name="inv2", tag="inv2")
                nc.sync.dma_start(inv2_t[:], inv2d[:])
                b2p_t = cpool.tile([128, NCH], f32, name="b2p", tag="b2p")
                nc.sync.dma_start(b2p_t[:], b2pd[:])
                w2T_t = cpool.tile([A, C], bf16, name="w2T", tag="w2T")
                nc.sync.dma_start(w2T_t[:], w2T[:])
                ident_t = cpool.tile([128, 128], f32, name="ident", tag="ident")
                nc.sync.dma_start(ident_t[:], identd[:])
                wt = cpool.tile([128, 2 * NCH * A], f32, name="wmsall", tag="wmsall")
                src_ap = wmsT.rearrange("(k p) a -> p k a", p=128)
                nc.sync.dma_start(wt[:].rearrange("p (k a) -> p k a", a=A), src_ap)
                wms_t = [wt[:, k * A:(k + 1) * A] for k in range(2 * NCH)]

            def body():
                global w1xT_t
                # prologue: phase A of samples 0/1, weights interleaved,
                # sample 2's DMA prefetched.  sum(x^2) rotates over
                # DVE/ACT/Pool so no engine serializes the (un-overlapped)
                # prologue.
                dma_x(0, groups=[0], split_first=True)
                w1xT_t = load_w1xT()
                dma_x(0, groups=[1, 2, 3])
                for c in range(NCH):
                    phaseA_moments(0, c, sq=PRO_SQ[c])
                    phaseA_mm1(0, c)
                load_params()
                dma_x(1)
                for c in range(NCH):
                    phaseA_moments(1, c, sq=PRO_SQ[(c + 6) % NCH])
                dma_x(2)
                phaseB(0)
                for c in range(NCH):
                    phaseA_mm1(1, c)
                # steady state: C(s) carries A(s+2) moments, B(s+1) spread
                # over c3/c4/c5/c8, and A(s+2)'s matmul1 in the c>=8 shadow
                # of relu(s+1) freeing the ph1 slot.  ACT's exp runs one
                # chunk ahead of the DVE; S2 accumulation runs one chunk
                # behind (its qt comes from the Pool engine).
                # (s, c) pairs for the exp-lookahead stream, two chunks ahead
                # of the DVE consumer.  g(s) is ready by c==8 of C(s-1), so
                # (s+1, 0) may be emitted from c==10 of C(s) onward.
                mm2exp_seq = [(s, c) for s in range(SPC) for c in range(NCH)]
                mm2exp_pos = 0

                def emit_mm2exp_upto(i):
                    nonlocal mm2exp_pos
                    while mm2exp_pos <= i and mm2exp_pos < len(mm2exp_seq):
                        phaseC_mm2exp(*mm2exp_seq[mm2exp_pos])
                        mm2exp_pos += 1

                emit_mm2exp_upto(1)
                for s in range(SPC):
                    has_a = s + 2 < SPC          # phase-A work interleaved?
                    qt_map = FRONT_QT if has_a else TAIL_QT
                    if s == SPC - 1:
                        qt_map = qt_map[:NCH - 2] + "DD"
                    s1_map = FRONT_S1 if has_a else TAIL_S1
                    for c in range(NCH):
                        emit_mm2exp_upto(s * NCH + c + 2)
                        phaseC_dve(s, c, qt_pool=(qt_map[c] == "P"),
                                   s1_act=(s1_map[c] == "A"))
                        if c > 1:
                            phaseC_s2(s, c - 2)
                        # deferred flush of the previous sample: its last S2s
                        # and the whole output stage run here, where the
                        # Pool's qt and the serial small-op chain can hide
                        # under this sample's chunk stream.
                        if c == 0 and s - 1 in st:
                            phaseC_s2(s - 1, NCH - 2)
                            sample_out_mu(s - 1)
                        if c == 1 and s - 1 in st:
                            phaseC_s2(s - 1, NCH - 1)
                        if c == 2 and s - 1 in st:
                            sample_out_sg(s - 1)
                            del st[s - 1]
                        if s + 3 < SPC and c == 0:
                            dma_x(s + 3)
                        if s == 2 and c < 4:
                            phaseA_moments(3, 8 + c, sq=FRONT_SQ[8 + c])
                        if has_a:
                            if s == 0:
                                # x(2)'s DMA was only issued in the prologue:
                                # shift A(2) two chunks later so the in-order
                                # DVE stream doesn't park on it.
                                if c >= 2:
                                    phaseA_moments(2, c - 2, sq=FRONT_SQ[c - 2])
                            elif s == 1:
                                # A(3) spills 4 chunks into C(2) to even the
                                # front/tail engine loads.
                                if c < 8:
                                    phaseA_moments(3, c, sq=FRONT_SQ[c])
                                if c < 2:
                                    phaseA_moments(2, 10 + c, sq=FRONT_SQ[10 + c])
                            else:
                                phaseA_moments(s + 2, c, sq=FRONT_SQ[c])
                        bc = (5, 6, 7, 8) if s == 2 else (3, 4, 5, 8)
                        if c == bc[0] and s + 1 < SPC:
                            phaseB_stats(s + 1)
                        if c == bc[1] and s + 1 < SPC:
                            phaseB_matvec2(s + 1)
                        if c == bc[2] and s + 1 < SPC:
                            phaseB_relu(s + 1)
                        if c == bc[3] and s + 1 < SPC:
                            phaseB_tanh(s + 1)
                        # mm1(s+2) reuses the single ph1 bank; its first
                        # matmul must be emitted after relu(s+1) has read it.
                        if has_a and c >= 8:
                            for cc in range(3 * (c - 8), 3 * (c - 8) + 3):
                                phaseA_mm1(s + 2, cc)
                s = SPC - 1
                phaseC_s2(s, NCH - 2)
                sample_out_mu(s)
                phaseC_s2(s, NCH - 1)
                sample_out_sg(s, iters=1)
                del st[s]

            if loop_reps == 1:
                body()
            else:
                with tc.For_i(0, loop_reps, 1):
                    body()

    nc.compile()
    return nc


def _get_module(loop_reps=1):
    key = loop_reps
    if key not in _CACHE:
        _CACHE[key] = _build_module(loop_reps)
    return _CACHE[key]


def _host_prep(inputs):
    """Precompute folded parameters and shard inputs. Returns per-core in_maps."""
    x = np.asarray(inputs["x"])
    W1 = np.asarray(inputs["W1"], np.float32)
    b1 = np.asarray(inputs["b1"], np.float32)
    g1 = np.asarray(inputs["g1"], np.float32)
    beta1 = np.asarray(inputs["beta1"], np.float32)
    rm1 = np.asarray(inputs["rm1"], np.float32)
    rv1 = np.asarray(inputs["rv1"], np.float32)
    W2 = np.asarray(inputs["W2"], np.float32)
    b2 = np.asarray(inputs["b2"], np.float32)
    g2 = np.asarray(inputs["g2"], np.float32)
    rv2 = np.asarray(inputs["rv2"], np.float32)

    inv1 = (g1 / np.sqrt(rv1 + BN_EPS)).astype(np.float32)
    add1 = (beta1 - rm1 * inv1).astype(np.float32)
    inv2 = (g2 / np.sqrt(rv2 + BN_EPS)).astype(np.float32)
    b2p = (inv2 * b2).astype(np.float32)

    const = {
        "w1xT": np.ascontiguousarray(W1[:, :C].T).astype(ml_dtypes.bfloat16),
        "wmsT": np.ascontiguousarray(W1[:, C:].T).astype(np.float32),
        "w2T": np.ascontiguousarray(W2.T).astype(ml_dtypes.bfloat16),
        "b1d": b1.reshape(A, 1),
        "inv1d": inv1.reshape(A, 1),
        "add1d": add1.reshape(A, 1),
        "inv2d": np.ascontiguousarray(inv2.reshape(NCH, 128).T),
        "b2pd": np.ascontiguousarray(b2p.reshape(NCH, 128).T),
        "identd": np.eye(128, dtype=np.float32),
    }
    xbf = x.astype(ml_dtypes.bfloat16)
    in_maps = []
    for core in range(N_CORES):
        m = dict(const)
        m["xbf"] = np.ascontiguousarray(xbf[core * SPC:(core + 1) * SPC])
        in_maps.append(m)
    return in_maps


def kernel(**inputs):
    from concourse.bass_utils import run_bass_kernel_spmd

    nc = _get_module(loop_reps=1)
    in_maps = _host_prep(inputs)
    res = run_bass_kernel_spmd(nc, in_maps, core_ids=list(range(N_CORES)))
    out = np.concatenate([res.results[i]["out"] for i in range(N_CORES)], axis=0)
    return out.astype(np.float32)
